# revision 18
# baseline (speedup 1.0000x reference)
"""Trainium2 Bass kernel for CNN_Text-style LSTM classifier.

Model: embedding lookup -> 512-step LSTM -> attention pooling -> FC -> softmax.
Strategy: data-parallel over batch (B=64 -> 8 cores x 8). All parameters
replicated. Per core, tokens are ordered seq-major: t = s*BL + b.

kernel(**inputs) takes FULL numpy inputs (as produced by setup_inputs) and
returns the FULL [64, 10] float32 output.
"""
import numpy as np
import ml_dtypes

import concourse.bass as bass
import concourse.tile as tile
from concourse import bacc, masks, mybir
from concourse.bass_utils import run_bass_kernel_spmd

BF16 = mybir.dt.bfloat16
F32 = mybir.dt.float32
I32 = mybir.dt.int32

# Full-problem constants
V, D, Co, C = 50000, 512, 512, 10
B, S = 64, 512
NCORES = 8
BL = B // NCORES          # local batch per core
G4 = 4 * Co               # 2048 gate dim
KC = D // 128             # 4 contraction chunks (D == Co == 512)
MC = G4 // 128            # 16 gate-dim chunks

SIG = mybir.ActivationFunctionType.Sigmoid
TANH = mybir.ActivationFunctionType.Tanh
EXP = mybir.ActivationFunctionType.Exp
IDENT = mybir.ActivationFunctionType.Identity
AX_X = mybir.AxisListType.X
ALU = mybir.AluOpType


def build_body(tc, io, S=S, V=V, rec_repeat=1, g_repeat=1, p2_repeat=1, p4_repeat=1, whh_fp8=False, split_o=True, free_run=False):
    """Emit the whole per-core program. io: dict of dram APs."""
    nc = tc.nc
    NTOK = S * BL
    NROWT = NTOK // 128      # gather row-tiles
    TT = min(512, NTOK)      # token tile for phase2/4 GEMMs
    NTT = NTOK // TT         # number of token tiles
    SPT = TT // BL           # steps per token tile

    idx_d = io["idx"]; embed_d = io["embed"]
    wihT_d = io["wihT"]; whhT_d = io["whhT"]; biasg_d = io["biasg"]
    wword_d = io["wword"]; bword_d = io["bword"]; wproj_d = io["wproj"]
    fcwT_d = io["fcwT"]; fcb_d = io["fcb"]; out_d = io["probs"]

    NTT0 = NTOK // min(512, NTOK)
    # xg layout: [partition, m-chunk, step-in-block, batch] — per-partition
    # contiguous for both the phase-2 writes (one m slice) and the phase-3
    # block reads (whole tensor), so DMA descriptors stay 2KB+, not 32B.
    xg_drams = [nc.dram_tensor("xg_scr%d" % i,
                               [128, MC, S // NTT0, BL], F32,
                               kind="Internal").ap() for i in range(NTT0)]
    scr_dram = nc.dram_tensor("sc_scr", [NTOK], F32, kind="Internal").ap()
    attn_dram = nc.dram_tensor("at_scr", [NTOK], F32, kind="Internal").ap()

    from contextlib import ExitStack
    _stack = ExitStack()
    const = _stack.enter_context(tc.tile_pool(name="const", bufs=1))
    state = _stack.enter_context(tc.tile_pool(name="state", bufs=1))

    # ---- constants to SBUF ----
    biasg_sb = const.tile([128, MC], F32)
    nc.sync.dma_start(biasg_sb, biasg_d.rearrange("(m p) -> p m", p=128))
    wword_sb = const.tile([128, KC, Co], BF16)
    nc.sync.dma_start(wword_sb, wword_d.rearrange("(k p) j -> p k j", p=128))
    bword_sb = const.tile([128, KC], F32)
    nc.sync.dma_start(bword_sb, bword_d.rearrange("(m p) -> p m", p=128))
    wproj_sb = const.tile([128, KC, 1], BF16)
    nc.sync.dma_start(wproj_sb, wproj_d.rearrange("(m p) o -> p m o", p=128))
    fcwT_sb = const.tile([128, KC, C], F32)
    nc.sync.dma_start(fcwT_sb, fcwT_d.rearrange("(k p) c -> p k c", p=128))
    fcb_bc = const.tile([BL, C], F32)
    nc.sync.dma_start(
        fcb_bc, bass.AP(tensor=fcb_d.tensor, offset=0, ap=[[0, BL], [1, C]]))
    idx_sb = const.tile([128, NROWT], I32)
    nc.sync.dma_start(idx_sb, idx_d.rearrange("(j p) -> p j", p=128))
    hzero = const.tile([128, KC, BL], BF16)
    nc.vector.memset(hzero, 0.0)
    ident = const.tile([128, 128], BF16)
    masks.make_identity(nc, ident[:])

    # ---- persistent state ----
    hr_all = state.tile([128, KC, NTOK], BF16)   # relu(h), transposed layout
    cT = state.tile([128, KC, BL], F32)
    nc.vector.memset(cT, 0.0)
    scores_sb = state.tile([1, NTOK], F32)
    ctxT_sb = state.tile([128, KC, BL], F32)

    # ================= Phase 1: gather + on-chip transpose =======
    # Gather 128-token row tiles, PE-transpose each 128x128 block, keep
    # e^T resident in SBUF (no DRAM roundtrip, no transpose-DMA).
    with tc.tile_pool(name="etsb", bufs=1) as etpool:
      eT_sb = etpool.tile([128, KC, NTOK], BF16)
      with nc.named_scope("p1_gather"), \
           tc.tile_pool(name="gat", bufs=4) as gpool, \
           tc.tile_pool(name="ps1", bufs=4, space="PSUM") as ps1pool:
        for _grep in range(g_repeat):
          for j in range(NROWT):
            g_sb = gpool.tile([128, D], BF16)
            nc.gpsimd.indirect_dma_start(
                out=g_sb[:], out_offset=None, in_=embed_d[:],
                in_offset=bass.IndirectOffsetOnAxis(ap=idx_sb[:, j:j + 1], axis=0))
            for k in range(KC):
                pt = ps1pool.tile([128, 128], BF16)
                nc.tensor.transpose(pt, g_sb[:, k * 128:(k + 1) * 128], ident)
                nc.vector.tensor_copy(eT_sb[:, k, j * 128:(j + 1) * 128], pt)

      # ================= Phase 2: xg = eT @ WihT + bias ============
      with nc.named_scope("p2_xg"), \
           tc.tile_pool(name="xout", bufs=4) as xopool, \
           tc.tile_pool(name="wih", bufs=1) as wihpool, \
           tc.tile_pool(name="ps2", bufs=6, space="PSUM") as ps2pool:
        wihT_sb = wihpool.tile([128, KC, G4], BF16)
        nc.sync.dma_start(wihT_sb, wihT_d.rearrange("(k p) g -> p k g", p=128))
        for _p2rep in range(p2_repeat):
         for nt in range(NTT):
            for m in range(MC):
                ps = ps2pool.tile([128, TT], F32)
                for k in range(KC):
                    nc.tensor.matmul(ps, wihT_sb[:, k, m * 128:(m + 1) * 128],
                                     eT_sb[:, k, nt * TT:(nt + 1) * TT],
                                     start=(k == 0), stop=(k == KC - 1))
                xsb = xopool.tile([128, SPT, BL], F32)
                nc.scalar.activation(xsb.rearrange("p a b -> p (a b)"), ps, IDENT,
                                     bias=biasg_sb[:, m:m + 1], scale=1.0)
                nc.sync.dma_start(xg_drams[nt][:, m, :, :], xsb)

    # ================= Phase 3: LSTM recurrence ==================
    # Per-step structure (gate blocks in torch order i=m0-3 f=m4-7 g=m8-11
    # o=m12-15): issue MMs for (i,f) then g then o; batched descale+activation
    # per block (one sigmoid over i,f; one tanh for g; one sigmoid for o);
    # c-chain runs during o's MMs; tail after last MM = tanh(c) then h-mul.
    with nc.named_scope("p3_lstm"), \
         tc.tile_pool(name="xstr", bufs=2) as xstream, \
         tc.tile_pool(name="gsb", bufs=3) as gpool3, \
         tc.tile_pool(name="tmp3", bufs=3) as tpool, \
         tc.tile_pool(name="hrot", bufs=3) as hpool, \
         tc.tile_pool(name="whh", bufs=1) as whhpool, \
         tc.tile_pool(name="ps3", bufs=2, space="PSUM") as ps3pool:
        if rec_repeat == 0:
            nc.vector.memset(hr_all, 0.0)
        else:
            whhT_sb = whhpool.tile([128, KC, G4],
                                   mybir.dt.float8e4 if whh_fp8 else BF16)
            nc.sync.dma_start(whhT_sb, whhT_d.rearrange("(k p) g -> p k g", p=128))

        def descale_add(out, ps, xg_slice):
            if whh_fp8:
                nc.vector.scalar_tensor_tensor(
                    out, ps, 0.125, xg_slice, op0=ALU.mult, op1=ALU.add)
            else:
                nc.vector.tensor_add(out, ps, xg_slice)

        for rep in range(rec_repeat):
          if rep > 0:
            nc.vector.memset(cT, 0.0)
          h_prev = hzero

          def fetch_block(nt):
              t = xstream.tile([128, MC, SPT, BL], F32, tag="xgblk")
              nc.sync.dma_start(t, xg_drams[nt])
              return t

          cur_blk = fetch_block(0)
          nxt_blk = None
          for s in range(S):
              if s % SPT == 0:
                  if s > 0:
                      cur_blk = nxt_blk
                  if s // SPT + 1 < NTT0:
                      nxt_blk = fetch_block(s // SPT + 1)
              xg_t = cur_blk[:, :, s % SPT, :]
              # 4 gate groups, issued f -> i -> g -> o so each gate's
              # descale+activation starts right after its own 16 MMs; the
              # c-chain overlaps the o MMs and the tail is just
              # sigmoid(o) || tanh(c) -> h-mul.
              ps_f = ps3pool.tile([128, 4, BL], F32, tag="ps_f")
              ps_i = ps3pool.tile([128, 4, BL], F32, tag="ps_i")
              ps_g = ps3pool.tile([128, 4, BL], F32, tag="ps_g")
              ps_o = ps3pool.tile([128, 4, BL], F32, tag="ps_o")
              for ps, m0 in ((ps_f, 4), (ps_i, 0), (ps_g, 8), (ps_o, 12)):
                  for j in range(4):
                      m = m0 + j
                      for k in range(KC):
                          nc.tensor.matmul(ps[:, j, :],
                                           whhT_sb[:, k, m * 128:(m + 1) * 128],
                                           hzero[:, k, :] if free_run
                                           else h_prev[:, k, :],
                                           start=(k == 0), stop=(k == KC - 1))
              # DVE order: all 4 descales first (none blocks the c-chain in
              # the FIFO), then fc, ig, c_add, h_mul. ACT: sig(i,f) merged,
              # tanh_g, sig_o, tanh_c.
              gfi = gpool3.tile([128, 8, BL], F32, tag="gfi")
              descale_add(gfi[:, 4:8, :], ps_f, xg_t[:, 4:8, :])
              descale_add(gfi[:, 0:4, :], ps_i, xg_t[:, 0:4, :])
              gg = gpool3.tile([128, 4, BL], F32, tag="gg")
              descale_add(gg, ps_g, xg_t[:, 8:12, :])
              go = gpool3.tile([128, 4, BL], F32, tag="go")
              descale_add(go, ps_o, xg_t[:, 12:16, :])
              nc.scalar.activation(gfi, gfi, SIG)
              nc.scalar.activation(gg, gg, TANH)
              nc.vector.tensor_mul(cT, gfi[:, 4:8, :], cT)   # f*c
              ig = tpool.tile([128, 4, BL], F32, tag="ig")
              nc.vector.tensor_mul(ig, gfi[:, 0:4, :], gg)
              nc.vector.tensor_add(cT, cT, ig)
              nc.scalar.activation(go, go, SIG)       # queued before tanh_c
              th = tpool.tile([128, 4, BL], F32, tag="th")
              nc.scalar.activation(th, cT, TANH)
              h_t = hpool.tile([128, KC, BL], BF16, tag="h")
              nc.vector.tensor_mul(h_t, go, th)
              nc.scalar.activation(hr_all[:, :, s * BL:(s + 1) * BL],
                                   h_t, mybir.ActivationFunctionType.Relu)
              h_prev = h_t

    # ================= Phase 4: attention + FC + softmax =========
    with nc.named_scope("p4_attn"), \
         tc.tile_pool(name="sq", bufs=2) as sqpool, \
         tc.tile_pool(name="p4", bufs=4) as p4pool, \
         tc.tile_pool(name="wh", bufs=1) as whpool, \
         tc.tile_pool(name="ps4", bufs=4, space="PSUM") as ps4pool, \
         tc.tile_pool(name="ps4b", bufs=2, space="PSUM") as ps4bpool:
      for _p4rep in range(p4_repeat):
        for nt in range(NTT):
            sq_tiles = []
            for mo in range(KC):
                ps = ps4pool.tile([128, TT], F32)
                for k in range(KC):
                    nc.tensor.matmul(ps, wword_sb[:, k, mo * 128:(mo + 1) * 128],
                                     hr_all[:, k, nt * TT:(nt + 1) * TT],
                                     start=(k == 0), stop=(k == KC - 1))
                sq = sqpool.tile([128, TT], BF16, tag=f"sq{mo}")
                nc.scalar.activation(sq, ps, TANH, bias=bword_sb[:, mo:mo + 1],
                                     scale=1.0)
                sq_tiles.append(sq)
            ps2 = ps4bpool.tile([1, TT], F32)
            for mo in range(KC):
                nc.tensor.matmul(ps2, wproj_sb[:, mo, :], sq_tiles[mo],
                                 start=(mo == 0), stop=(mo == KC - 1))
            nc.vector.tensor_copy(scores_sb[0:1, nt * TT:(nt + 1) * TT], ps2)

        # softmax over sequence, per batch element
        nc.sync.dma_start(scr_dram.rearrange("(o t) -> o t", o=1), scores_sb)
        sc_bs = p4pool.tile([BL, S], F32)
        nc.sync.dma_start(sc_bs, scr_dram.rearrange("(s b) -> b s", b=BL))
        mx = p4pool.tile([BL, 1], F32)
        nc.vector.tensor_reduce(mx, sc_bs, axis=AX_X, op=ALU.max)
        nc.vector.tensor_scalar_mul(mx, mx, -1.0)
        at = p4pool.tile([BL, S], F32)
        nc.scalar.activation(at, sc_bs, EXP, bias=mx[:, 0:1], scale=1.0)
        sm = p4pool.tile([BL, 1], F32)
        nc.vector.tensor_reduce(sm, at, axis=AX_X, op=ALU.add)
        nc.vector.reciprocal(sm, sm)
        nc.vector.tensor_scalar_mul(at, at, sm)
        nc.sync.dma_start(attn_dram.rearrange("(s b) -> b s", b=BL), at)
        attn_bc = whpool.tile([128, NTOK], F32, tag="abc")
        nc.sync.dma_start(
            attn_bc,
            bass.AP(tensor=attn_dram.tensor, offset=0, ap=[[0, 128], [1, NTOK]]))

        # ctx = sum_s attn * relu(h)
        for ch in range(KC):
            wh = whpool.tile([128, NTOK], F32, tag="wh")
            nc.vector.tensor_mul(wh, hr_all[:, ch, :], attn_bc)
            nc.vector.tensor_reduce(ctxT_sb[:, ch, :],
                                    wh.rearrange("p (s b) -> p b s", b=BL),
                                    axis=AX_X, op=ALU.add)

        # logits + softmax
        psL = ps4bpool.tile([BL, C], F32)
        for ch in range(KC):
            nc.tensor.matmul(psL, ctxT_sb[:, ch, :], fcwT_sb[:, ch, :],
                             start=(ch == 0), stop=(ch == KC - 1))
        lg = p4pool.tile([BL, C], F32)
        nc.vector.tensor_add(lg, psL, fcb_bc)
        mx2 = p4pool.tile([BL, 1], F32)
        nc.vector.tensor_reduce(mx2, lg, axis=AX_X, op=ALU.max)
        nc.vector.tensor_scalar_mul(mx2, mx2, -1.0)
        pe = p4pool.tile([BL, C], F32)
        nc.scalar.activation(pe, lg, EXP, bias=mx2[:, 0:1], scale=1.0)
        sm2 = p4pool.tile([BL, 1], F32)
        nc.vector.tensor_reduce(sm2, pe, axis=AX_X, op=ALU.add)
        nc.vector.reciprocal(sm2, sm2)
        nc.vector.tensor_scalar_mul(pe, pe, sm2)
        nc.sync.dma_start(out_d, pe)
    _stack.close()


def build_nc(S=S, V=V, **bkw):
    nc = bacc.Bacc("TRN2", target_bir_lowering=False, debug=False,
                   num_devices=NCORES)
    NTOK = S * BL
    whh_dt = mybir.dt.float8e4 if bkw.get("whh_fp8") else BF16
    io = {
        "idx": nc.dram_tensor("idx", [NTOK], I32, kind="ExternalInput").ap(),
        "embed": nc.dram_tensor("embed", [V, D], BF16, kind="ExternalInput").ap(),
        "wihT": nc.dram_tensor("wihT", [D, G4], BF16, kind="ExternalInput").ap(),
        "whhT": nc.dram_tensor("whhT", [Co, G4], whh_dt, kind="ExternalInput").ap(),
        "biasg": nc.dram_tensor("biasg", [G4], F32, kind="ExternalInput").ap(),
        "wword": nc.dram_tensor("wword", [Co, Co], BF16, kind="ExternalInput").ap(),
        "bword": nc.dram_tensor("bword", [Co], F32, kind="ExternalInput").ap(),
        "wproj": nc.dram_tensor("wproj", [Co, 1], BF16, kind="ExternalInput").ap(),
        "fcwT": nc.dram_tensor("fcwT", [Co, C], F32, kind="ExternalInput").ap(),
        "fcb": nc.dram_tensor("fcb", [C], F32, kind="ExternalInput").ap(),
        "probs": nc.dram_tensor("probs", [BL, C], F32, kind="ExternalOutput").ap(),
    }
    with tile.TileContext(nc) as tc:
        build_body(tc, io, S=S, V=V, **bkw)
    nc.compile()
    return nc


def host_prep(inputs, whh_fp8=False):
    """Cast/transpose parameters on host; build per-core in_maps."""
    bf = ml_dtypes.bfloat16
    x = np.asarray(inputs["x"])
    common = {
        "embed": np.ascontiguousarray(np.asarray(inputs["embed"]).astype(bf)),
        "wihT": np.ascontiguousarray(np.asarray(inputs["W_ih"]).T.astype(bf)),
        "whhT": (np.ascontiguousarray((np.asarray(inputs["W_hh"]).T * 8.0).astype(ml_dtypes.float8_e4m3fn))
                  if whh_fp8 else
                  np.ascontiguousarray(np.asarray(inputs["W_hh"]).T.astype(bf))),
        "biasg": np.ascontiguousarray(
            (np.asarray(inputs["b_ih"]) + np.asarray(inputs["b_hh"])).astype(np.float32)),
        "wword": np.ascontiguousarray(np.asarray(inputs["weight_word"]).astype(bf)),
        "bword": np.ascontiguousarray(np.asarray(inputs["bias_word"])[:, 0].astype(np.float32)),
        "wproj": np.ascontiguousarray(np.asarray(inputs["weight_proj_word"]).astype(bf)),
        "fcwT": np.ascontiguousarray(np.asarray(inputs["fc_w"]).T.astype(np.float32)),
        "fcb": np.ascontiguousarray(np.asarray(inputs["fc_b"]).astype(np.float32)),
    }
    in_maps = []
    for c in range(NCORES):
        shard = x[c * BL:(c + 1) * BL, :]          # [BL, S]
        idx = np.ascontiguousarray(shard.T.reshape(-1).astype(np.int32))  # s-major
        in_maps.append({"idx": idx, **common})
    return in_maps


_NC_CACHE = {}
WHH_FP8 = True


def _get_nc():
    if "nc" not in _NC_CACHE:
        _NC_CACHE["nc"] = build_nc(whh_fp8=WHH_FP8)
    return _NC_CACHE["nc"]


def kernel(**inputs):
    nc = _get_nc()
    in_maps = host_prep(inputs, whh_fp8=WHH_FP8)
    res = run_bass_kernel_spmd(nc, in_maps, core_ids=list(range(NCORES)))
    probs = np.concatenate([res.results[c]["probs"] for c in range(NCORES)], axis=0)
    return probs.astype(np.float32)


def run_traced(inputs):
    """Like kernel() but with NTFF tracing; returns (probs, BassKernelResults)."""
    nc = _get_nc()
    in_maps = host_prep(inputs, whh_fp8=WHH_FP8)
    res = run_bass_kernel_spmd(nc, in_maps, core_ids=list(range(NCORES)),
                               trace=True)
    probs = np.concatenate([res.results[c]["probs"] for c in range(NCORES)], axis=0)
    return probs.astype(np.float32), res



# revision 23
# speedup vs baseline: 1.0578x; 1.0578x over previous
"""Trainium2 Bass kernel for CNN_Text-style LSTM classifier.

Model: embedding lookup -> 512-step LSTM -> attention pooling -> FC -> softmax.
Strategy: data-parallel over batch (B=64 -> 8 cores x 8). All parameters
replicated. Per core, tokens are ordered seq-major: t = s*BL + b.

kernel(**inputs) takes FULL numpy inputs (as produced by setup_inputs) and
returns the FULL [64, 10] float32 output.
"""
import numpy as np
import ml_dtypes

import concourse.bass as bass
import concourse.tile as tile
from concourse import bacc, masks, mybir
from concourse.bass_utils import run_bass_kernel_spmd

BF16 = mybir.dt.bfloat16
F32 = mybir.dt.float32
I32 = mybir.dt.int32

# Full-problem constants
V, D, Co, C = 50000, 512, 512, 10
B, S = 64, 512
NCORES = 8
BL = B // NCORES          # local batch per core
G4 = 4 * Co               # 2048 gate dim
KC = D // 128             # 4 contraction chunks (D == Co == 512)
MC = G4 // 128            # 16 gate-dim chunks

SIG = mybir.ActivationFunctionType.Sigmoid
TANH = mybir.ActivationFunctionType.Tanh
EXP = mybir.ActivationFunctionType.Exp
IDENT = mybir.ActivationFunctionType.Identity
AX_X = mybir.AxisListType.X
ALU = mybir.AluOpType


def build_body(tc, io, S=S, V=V, rec_repeat=1, g_repeat=1, p2_repeat=1, p4_repeat=1, whh_fp8=False, split_o=True, free_run=False):
    """Emit the whole per-core program. io: dict of dram APs."""
    nc = tc.nc
    NTOK = S * BL
    NROWT = NTOK // 128      # gather row-tiles
    TT = min(512, NTOK)      # token tile for phase2/4 GEMMs
    NTT = NTOK // TT         # number of token tiles
    SPT = TT // BL           # steps per token tile

    idx_d = io["idx"]; embed_d = io["embed"]
    wihT_d = io["wihT"]; whhT_d = io["whhT"]; biasg_d = io["biasg"]
    wword_d = io["wword"]; bword_d = io["bword"]; wproj_d = io["wproj"]
    fcwT_d = io["fcwT"]; fcb_d = io["fcb"]; out_d = io["probs"]

    NTT0 = NTOK // min(512, NTOK)
    scr_dram = nc.dram_tensor("sc_scr", [NTOK], F32, kind="Internal").ap()
    attn_dram = nc.dram_tensor("at_scr", [NTOK], F32, kind="Internal").ap()

    from contextlib import ExitStack
    _stack = ExitStack()
    const = _stack.enter_context(tc.tile_pool(name="const", bufs=1))
    state = _stack.enter_context(tc.tile_pool(name="state", bufs=1))

    # ---- constants to SBUF ----
    biasg_sb = const.tile([128, MC], F32)
    nc.sync.dma_start(biasg_sb, biasg_d.rearrange("(m p) -> p m", p=128))
    wword_sb = const.tile([128, KC, Co], BF16)
    nc.sync.dma_start(wword_sb, wword_d.rearrange("(k p) j -> p k j", p=128))
    bword_sb = const.tile([128, KC], F32)
    nc.sync.dma_start(bword_sb, bword_d.rearrange("(m p) -> p m", p=128))
    wproj_sb = const.tile([128, KC, 1], BF16)
    nc.sync.dma_start(wproj_sb, wproj_d.rearrange("(m p) o -> p m o", p=128))
    fcwT_sb = const.tile([128, KC, C], F32)
    nc.sync.dma_start(fcwT_sb, fcwT_d.rearrange("(k p) c -> p k c", p=128))
    fcb_bc = const.tile([BL, C], F32)
    nc.sync.dma_start(
        fcb_bc, bass.AP(tensor=fcb_d.tensor, offset=0, ap=[[0, BL], [1, C]]))
    idx_sb = const.tile([128, NROWT], I32)
    nc.sync.dma_start(idx_sb, idx_d.rearrange("(j p) -> p j", p=128))
    hzero = const.tile([128, KC, BL], BF16)
    nc.vector.memset(hzero, 0.0)
    ident = const.tile([128, 128], BF16)
    masks.make_identity(nc, ident[:])
    wihT_sb = const.tile([128, KC, G4], BF16)
    nc.sync.dma_start(wihT_sb, wihT_d.rearrange("(k p) g -> p k g", p=128))

    # ---- persistent state ----
    hr_all = state.tile([128, KC, NTOK], BF16)   # relu(h), transposed layout
    cT = state.tile([128, KC, BL], F32)
    nc.vector.memset(cT, 0.0)
    scores_sb = state.tile([1, NTOK], F32)
    ctxT_sb = state.tile([128, KC, BL], F32)

    # ================= Phase 1: gather + on-chip transpose =======
    # Gather 128-token row tiles, PE-transpose each 128x128 block, keep
    # e^T resident in SBUF (no DRAM roundtrip, no transpose-DMA).
    with tc.tile_pool(name="etsb", bufs=1) as etpool:
      eT_sb = etpool.tile([128, KC, NTOK], BF16)
      with nc.named_scope("p1_gather"), \
           tc.tile_pool(name="gat", bufs=4) as gpool, \
           tc.tile_pool(name="ps1", bufs=4, space="PSUM") as ps1pool:
        for _grep in range(g_repeat):
          for j in range(NROWT):
            g_sb = gpool.tile([128, D], BF16)
            nc.gpsimd.indirect_dma_start(
                out=g_sb[:], out_offset=None, in_=embed_d[:],
                in_offset=bass.IndirectOffsetOnAxis(ap=idx_sb[:, j:j + 1], axis=0))
            for k in range(KC):
                pt = ps1pool.tile([128, 128], BF16)
                nc.tensor.transpose(pt, g_sb[:, k * 128:(k + 1) * 128], ident)
                nc.vector.tensor_copy(eT_sb[:, k, j * 128:(j + 1) * 128], pt)

      # ================= Phase 2+3 fused ===========================
      # xg for token-tile nt lives in an SBUF block tile [128, MC, SPT, BL]
      # (no DRAM roundtrip). Tile 0 is produced in a prologue; tile nt+1's
      # 16 (m)-groups (4 MMs + 1 psum->SBUF eviction w/ bias) are spread one
      # per 4 steps of block nt, filling the PE/ACT idle in the step tail.
      # ================= Phase 3: LSTM recurrence (p2 fused in) ====
      # Per-step: 4 gate groups f -> i -> g -> o, descale+activation per
      # group; c-chain overlaps o MMs; tail = sigmoid(o) || tanh(c) -> h.
      # Tile nt+1's xg m-groups (4 MMs + biased eviction into an SBUF block
      # tile) are emitted one per 4 steps, so they execute in the PE/ACT
      # idle of the step tail. Tile 0 is a prologue.
      with nc.named_scope("p3_lstm"), \
           tc.tile_pool(name="xstr", bufs=2) as xstream, \
           tc.tile_pool(name="gsb", bufs=3) as gpool3, \
           tc.tile_pool(name="tmp3", bufs=3) as tpool, \
           tc.tile_pool(name="hrot", bufs=3) as hpool, \
           tc.tile_pool(name="whh", bufs=1) as whhpool, \
           tc.tile_pool(name="ps2", bufs=2, space="PSUM") as ps2pool, \
           tc.tile_pool(name="ps3", bufs=1, space="PSUM") as ps3pool:
        if rec_repeat == 0:
            nc.vector.memset(hr_all, 0.0)
        else:
            whhT_sb = whhpool.tile([128, KC, G4],
                                   mybir.dt.float8e4 if whh_fp8 else BF16)
            nc.sync.dma_start(whhT_sb, whhT_d.rearrange("(k p) g -> p k g", p=128))

        def descale_add(out, ps, xg_slice):
            if whh_fp8:
                nc.vector.scalar_tensor_tensor(
                    out, ps, 0.125, xg_slice, op0=ALU.mult, op1=ALU.add)
            else:
                nc.vector.tensor_add(out, ps, xg_slice)

        def emit_p2_group(dst_blk, nt, m):
            ps = ps2pool.tile([128, TT], F32)
            for k in range(KC):
                nc.tensor.matmul(ps, wihT_sb[:, k, m * 128:(m + 1) * 128],
                                 eT_sb[:, k, nt * TT:(nt + 1) * TT],
                                 start=(k == 0), stop=(k == KC - 1))
            nc.scalar.activation(
                dst_blk[:, m, :, :].rearrange("p a b -> p (a b)"), ps, IDENT,
                bias=biasg_sb[:, m:m + 1], scale=1.0)

        for rep in range(rec_repeat):
          if rep > 0:
            nc.vector.memset(cT, 0.0)
          h_prev = hzero
          with nc.named_scope("p2_xg"):
              cur_blk = xstream.tile([128, MC, SPT, BL], F32, tag="xgblk")
              for m in range(MC):
                  emit_p2_group(cur_blk, 0, m)
          nxt_blk = None
          for s in range(S):
              nt = s // SPT
              if s % SPT == 0:
                  if s > 0:
                      cur_blk = nxt_blk
                  if nt + 1 < NTT:
                      nxt_blk = xstream.tile([128, MC, SPT, BL], F32,
                                             tag="xgblk")
              if nt + 1 < NTT and s % 4 == 0:
                  emit_p2_group(nxt_blk, nt + 1, (s % SPT) // 4)
              xg_t = cur_blk[:, :, s % SPT, :]
              # 4 gate groups, issued f -> i -> g -> o so each gate's
              # descale+activation starts right after its own 16 MMs; the
              # c-chain overlaps the o MMs and the tail is just
              # sigmoid(o) || tanh(c) -> h-mul.
              ps_f = ps3pool.tile([128, 4, BL], F32, tag="ps_f")
              ps_i = ps3pool.tile([128, 4, BL], F32, tag="ps_i")
              ps_g = ps3pool.tile([128, 4, BL], F32, tag="ps_g")
              ps_o = ps3pool.tile([128, 4, BL], F32, tag="ps_o")
              for ps, m0 in ((ps_f, 4), (ps_i, 0), (ps_g, 8), (ps_o, 12)):
                  for j in range(4):
                      m = m0 + j
                      for k in range(KC):
                          nc.tensor.matmul(ps[:, j, :],
                                           whhT_sb[:, k, m * 128:(m + 1) * 128],
                                           hzero[:, k, :] if free_run
                                           else h_prev[:, k, :],
                                           start=(k == 0), stop=(k == KC - 1))
              gf = gpool3.tile([128, 4, BL], F32, tag="gf")
              descale_add(gf, ps_f, xg_t[:, 4:8, :])
              nc.scalar.activation(gf, gf, SIG)
              gi = gpool3.tile([128, 4, BL], F32, tag="gi")
              descale_add(gi, ps_i, xg_t[:, 0:4, :])
              nc.scalar.activation(gi, gi, SIG)
              nc.vector.tensor_mul(cT, gf, cT)        # f*c during g/o MMs
              gg = gpool3.tile([128, 4, BL], F32, tag="gg")
              descale_add(gg, ps_g, xg_t[:, 8:12, :])
              nc.scalar.activation(gg, gg, TANH)
              go = gpool3.tile([128, 4, BL], F32, tag="go")
              descale_add(go, ps_o, xg_t[:, 12:16, :])
              ig = tpool.tile([128, 4, BL], F32, tag="ig")
              nc.vector.tensor_mul(ig, gi, gg)
              nc.vector.tensor_add(cT, cT, ig)
              nc.scalar.activation(go, go, SIG)       # queued before tanh_c
              th = tpool.tile([128, 4, BL], F32, tag="th")
              nc.scalar.activation(th, cT, TANH)
              h_t = hpool.tile([128, KC, BL], BF16, tag="h")
              nc.vector.tensor_mul(h_t, go, th)
              nc.vector.tensor_scalar_max(
                  hr_all[:, :, s * BL:(s + 1) * BL], h_t, 0.0)
              h_prev = h_t

    # ================= Phase 4: attention + FC + softmax =========
    with nc.named_scope("p4_attn"), \
         tc.tile_pool(name="sq", bufs=2) as sqpool, \
         tc.tile_pool(name="p4", bufs=4) as p4pool, \
         tc.tile_pool(name="wh", bufs=1) as whpool, \
         tc.tile_pool(name="ps4", bufs=4, space="PSUM") as ps4pool, \
         tc.tile_pool(name="ps4b", bufs=2, space="PSUM") as ps4bpool:
      for _p4rep in range(p4_repeat):
        for nt in range(NTT):
            sq_tiles = []
            for mo in range(KC):
                ps = ps4pool.tile([128, TT], F32)
                for k in range(KC):
                    nc.tensor.matmul(ps, wword_sb[:, k, mo * 128:(mo + 1) * 128],
                                     hr_all[:, k, nt * TT:(nt + 1) * TT],
                                     start=(k == 0), stop=(k == KC - 1))
                sq = sqpool.tile([128, TT], BF16, tag=f"sq{mo}")
                nc.scalar.activation(sq, ps, TANH, bias=bword_sb[:, mo:mo + 1],
                                     scale=1.0)
                sq_tiles.append(sq)
            ps2 = ps4bpool.tile([1, TT], F32)
            for mo in range(KC):
                nc.tensor.matmul(ps2, wproj_sb[:, mo, :], sq_tiles[mo],
                                 start=(mo == 0), stop=(mo == KC - 1))
            nc.vector.tensor_copy(scores_sb[0:1, nt * TT:(nt + 1) * TT], ps2)

        # softmax over sequence, per batch element
        nc.sync.dma_start(scr_dram.rearrange("(o t) -> o t", o=1), scores_sb)
        sc_bs = p4pool.tile([BL, S], F32)
        nc.sync.dma_start(sc_bs, scr_dram.rearrange("(s b) -> b s", b=BL))
        mx = p4pool.tile([BL, 1], F32)
        nc.vector.tensor_reduce(mx, sc_bs, axis=AX_X, op=ALU.max)
        nc.vector.tensor_scalar_mul(mx, mx, -1.0)
        at = p4pool.tile([BL, S], F32)
        nc.scalar.activation(at, sc_bs, EXP, bias=mx[:, 0:1], scale=1.0)
        sm = p4pool.tile([BL, 1], F32)
        nc.vector.tensor_reduce(sm, at, axis=AX_X, op=ALU.add)
        nc.vector.reciprocal(sm, sm)
        nc.vector.tensor_scalar_mul(at, at, sm)
        nc.sync.dma_start(attn_dram.rearrange("(s b) -> b s", b=BL), at)
        attn_bc = whpool.tile([128, NTOK], F32, tag="abc")
        nc.sync.dma_start(
            attn_bc,
            bass.AP(tensor=attn_dram.tensor, offset=0, ap=[[0, 128], [1, NTOK]]))

        # ctx = sum_s attn * relu(h)
        for ch in range(KC):
            wh = whpool.tile([128, NTOK], F32, tag="wh")
            nc.vector.tensor_mul(wh, hr_all[:, ch, :], attn_bc)
            nc.vector.tensor_reduce(ctxT_sb[:, ch, :],
                                    wh.rearrange("p (s b) -> p b s", b=BL),
                                    axis=AX_X, op=ALU.add)

        # logits + softmax
        psL = ps4bpool.tile([BL, C], F32)
        for ch in range(KC):
            nc.tensor.matmul(psL, ctxT_sb[:, ch, :], fcwT_sb[:, ch, :],
                             start=(ch == 0), stop=(ch == KC - 1))
        lg = p4pool.tile([BL, C], F32)
        nc.vector.tensor_add(lg, psL, fcb_bc)
        mx2 = p4pool.tile([BL, 1], F32)
        nc.vector.tensor_reduce(mx2, lg, axis=AX_X, op=ALU.max)
        nc.vector.tensor_scalar_mul(mx2, mx2, -1.0)
        pe = p4pool.tile([BL, C], F32)
        nc.scalar.activation(pe, lg, EXP, bias=mx2[:, 0:1], scale=1.0)
        sm2 = p4pool.tile([BL, 1], F32)
        nc.vector.tensor_reduce(sm2, pe, axis=AX_X, op=ALU.add)
        nc.vector.reciprocal(sm2, sm2)
        nc.vector.tensor_scalar_mul(pe, pe, sm2)
        nc.sync.dma_start(out_d, pe)
    _stack.close()


def build_nc(S=S, V=V, **bkw):
    nc = bacc.Bacc("TRN2", target_bir_lowering=False, debug=False,
                   num_devices=NCORES)
    NTOK = S * BL
    whh_dt = mybir.dt.float8e4 if bkw.get("whh_fp8") else BF16
    io = {
        "idx": nc.dram_tensor("idx", [NTOK], I32, kind="ExternalInput").ap(),
        "embed": nc.dram_tensor("embed", [V, D], BF16, kind="ExternalInput").ap(),
        "wihT": nc.dram_tensor("wihT", [D, G4], BF16, kind="ExternalInput").ap(),
        "whhT": nc.dram_tensor("whhT", [Co, G4], whh_dt, kind="ExternalInput").ap(),
        "biasg": nc.dram_tensor("biasg", [G4], F32, kind="ExternalInput").ap(),
        "wword": nc.dram_tensor("wword", [Co, Co], BF16, kind="ExternalInput").ap(),
        "bword": nc.dram_tensor("bword", [Co], F32, kind="ExternalInput").ap(),
        "wproj": nc.dram_tensor("wproj", [Co, 1], BF16, kind="ExternalInput").ap(),
        "fcwT": nc.dram_tensor("fcwT", [Co, C], F32, kind="ExternalInput").ap(),
        "fcb": nc.dram_tensor("fcb", [C], F32, kind="ExternalInput").ap(),
        "probs": nc.dram_tensor("probs", [BL, C], F32, kind="ExternalOutput").ap(),
    }
    with tile.TileContext(nc) as tc:
        build_body(tc, io, S=S, V=V, **bkw)
    nc.compile()
    return nc


def host_prep(inputs, whh_fp8=False):
    """Cast/transpose parameters on host; build per-core in_maps."""
    bf = ml_dtypes.bfloat16
    x = np.asarray(inputs["x"])
    common = {
        "embed": np.ascontiguousarray(np.asarray(inputs["embed"]).astype(bf)),
        "wihT": np.ascontiguousarray(np.asarray(inputs["W_ih"]).T.astype(bf)),
        "whhT": (np.ascontiguousarray((np.asarray(inputs["W_hh"]).T * 8.0).astype(ml_dtypes.float8_e4m3fn))
                  if whh_fp8 else
                  np.ascontiguousarray(np.asarray(inputs["W_hh"]).T.astype(bf))),
        "biasg": np.ascontiguousarray(
            (np.asarray(inputs["b_ih"]) + np.asarray(inputs["b_hh"])).astype(np.float32)),
        "wword": np.ascontiguousarray(np.asarray(inputs["weight_word"]).astype(bf)),
        "bword": np.ascontiguousarray(np.asarray(inputs["bias_word"])[:, 0].astype(np.float32)),
        "wproj": np.ascontiguousarray(np.asarray(inputs["weight_proj_word"]).astype(bf)),
        "fcwT": np.ascontiguousarray(np.asarray(inputs["fc_w"]).T.astype(np.float32)),
        "fcb": np.ascontiguousarray(np.asarray(inputs["fc_b"]).astype(np.float32)),
    }
    in_maps = []
    for c in range(NCORES):
        shard = x[c * BL:(c + 1) * BL, :]          # [BL, S]
        idx = np.ascontiguousarray(shard.T.reshape(-1).astype(np.int32))  # s-major
        in_maps.append({"idx": idx, **common})
    return in_maps


_NC_CACHE = {}
WHH_FP8 = True


def _get_nc():
    if "nc" not in _NC_CACHE:
        _NC_CACHE["nc"] = build_nc(whh_fp8=WHH_FP8)
    return _NC_CACHE["nc"]


def kernel(**inputs):
    nc = _get_nc()
    in_maps = host_prep(inputs, whh_fp8=WHH_FP8)
    res = run_bass_kernel_spmd(nc, in_maps, core_ids=list(range(NCORES)))
    probs = np.concatenate([res.results[c]["probs"] for c in range(NCORES)], axis=0)
    return probs.astype(np.float32)


def run_traced(inputs):
    """Like kernel() but with NTFF tracing; returns (probs, BassKernelResults)."""
    nc = _get_nc()
    in_maps = host_prep(inputs, whh_fp8=WHH_FP8)
    res = run_bass_kernel_spmd(nc, in_maps, core_ids=list(range(NCORES)),
                               trace=True)
    probs = np.concatenate([res.results[c]["probs"] for c in range(NCORES)], axis=0)
    return probs.astype(np.float32), res



# revision 27
# speedup vs baseline: 1.0648x; 1.0066x over previous
"""Trainium2 Bass kernel for CNN_Text-style LSTM classifier.

Model: embedding lookup -> 512-step LSTM -> attention pooling -> FC -> softmax.
Strategy: data-parallel over batch (B=64 -> 8 cores x 8). All parameters
replicated. Per core, tokens are ordered seq-major: t = s*BL + b.

kernel(**inputs) takes FULL numpy inputs (as produced by setup_inputs) and
returns the FULL [64, 10] float32 output.
"""
import numpy as np
import ml_dtypes

import concourse.bass as bass
import concourse.tile as tile
from concourse import bacc, masks, mybir
from concourse.bass_utils import run_bass_kernel_spmd

BF16 = mybir.dt.bfloat16
F32 = mybir.dt.float32
I32 = mybir.dt.int32

# Full-problem constants
V, D, Co, C = 50000, 512, 512, 10
B, S = 64, 512
NCORES = 8
BL = B // NCORES          # local batch per core
G4 = 4 * Co               # 2048 gate dim
KC = D // 128             # 4 contraction chunks (D == Co == 512)
MC = G4 // 128            # 16 gate-dim chunks

SIG = mybir.ActivationFunctionType.Sigmoid
TANH = mybir.ActivationFunctionType.Tanh
EXP = mybir.ActivationFunctionType.Exp
IDENT = mybir.ActivationFunctionType.Identity
AX_X = mybir.AxisListType.X
ALU = mybir.AluOpType


def build_body(tc, io, S=S, V=V, rec_repeat=1, g_repeat=1, p2_repeat=1, p4_repeat=1, whh_fp8=False, split_o=True, free_run=False):
    """Emit the whole per-core program. io: dict of dram APs."""
    nc = tc.nc
    NTOK = S * BL
    NROWT = NTOK // 128      # gather row-tiles
    TT = min(512, NTOK)      # token tile for phase2/4 GEMMs
    NTT = NTOK // TT         # number of token tiles
    SPT = TT // BL           # steps per token tile

    idx_d = io["idx"]; embed_d = io["embed"]
    wihT_d = io["wihT"]; whhT_d = io["whhT"]; biasg_d = io["biasg"]
    wword_d = io["wword"]; bword_d = io["bword"]; wproj_d = io["wproj"]
    fcwT_d = io["fcwT"]; fcb_d = io["fcb"]; out_d = io["probs"]

    NTT0 = NTOK // min(512, NTOK)
    scr_dram = nc.dram_tensor("sc_scr", [NTOK], F32, kind="Internal").ap()
    attn_dram = nc.dram_tensor("at_scr", [NTOK], F32, kind="Internal").ap()

    from contextlib import ExitStack
    _stack = ExitStack()
    const = _stack.enter_context(tc.tile_pool(name="const", bufs=1))
    state = _stack.enter_context(tc.tile_pool(name="state", bufs=1))

    # ---- constants to SBUF ----
    biasg_sb = const.tile([128, MC], F32)
    nc.sync.dma_start(biasg_sb, biasg_d.rearrange("(m p) -> p m", p=128))
    wword_sb = const.tile([128, KC, Co], BF16)
    nc.sync.dma_start(wword_sb, wword_d.rearrange("(k p) j -> p k j", p=128))
    bword_sb = const.tile([128, KC], F32)
    nc.sync.dma_start(bword_sb, bword_d.rearrange("(m p) -> p m", p=128))
    wproj_sb = const.tile([128, KC, 1], BF16)
    nc.sync.dma_start(wproj_sb, wproj_d.rearrange("(m p) o -> p m o", p=128))
    fcwT_sb = const.tile([128, KC, C], F32)
    nc.sync.dma_start(fcwT_sb, fcwT_d.rearrange("(k p) c -> p k c", p=128))
    fcb_bc = const.tile([BL, C], F32)
    nc.sync.dma_start(
        fcb_bc, bass.AP(tensor=fcb_d.tensor, offset=0, ap=[[0, BL], [1, C]]))
    idx_sb = const.tile([128, NROWT], I32)
    nc.sync.dma_start(idx_sb, idx_d.rearrange("(j p) -> p j", p=128))
    hzero = const.tile([128, KC, BL], BF16)
    nc.vector.memset(hzero, 0.0)
    ident = const.tile([128, 128], BF16)
    masks.make_identity(nc, ident[:])
    wihT_sb = const.tile([128, KC, G4], BF16)
    nc.sync.dma_start(wihT_sb, wihT_d.rearrange("(k p) g -> p k g", p=128))

    # ---- persistent state ----
    hr_all = state.tile([128, KC, NTOK], BF16)   # relu(h), transposed layout
    cT = state.tile([128, KC, BL], F32)
    nc.vector.memset(cT, 0.0)
    scores_sb = state.tile([1, NTOK], F32)
    ctxT_sb = state.tile([128, KC, BL], F32)

    # ================= Phase 1: gather + on-chip transpose =======
    # Gather 128-token row tiles, PE-transpose each 128x128 block, keep
    # e^T resident in SBUF (no DRAM roundtrip, no transpose-DMA).
    with tc.tile_pool(name="etsb", bufs=1) as etpool:
      eT_sb = etpool.tile([128, KC, NTOK], BF16)
      with nc.named_scope("p1_gather"), \
           tc.tile_pool(name="gat", bufs=4) as gpool, \
           tc.tile_pool(name="ps1", bufs=4, space="PSUM") as ps1pool:
        for _grep in range(g_repeat):
          for j in range(NROWT):
            g_sb = gpool.tile([128, D], BF16)
            nc.gpsimd.indirect_dma_start(
                out=g_sb[:], out_offset=None, in_=embed_d[:],
                in_offset=bass.IndirectOffsetOnAxis(ap=idx_sb[:, j:j + 1], axis=0))
            for k in range(KC):
                pt = ps1pool.tile([128, 128], BF16)
                nc.tensor.transpose(pt, g_sb[:, k * 128:(k + 1) * 128], ident)
                nc.vector.tensor_copy(eT_sb[:, k, j * 128:(j + 1) * 128], pt)

      # ================= Phase 2+3 fused ===========================
      # xg for token-tile nt lives in an SBUF block tile [128, MC, SPT, BL]
      # (no DRAM roundtrip). Tile 0 is produced in a prologue; tile nt+1's
      # 16 (m)-groups (4 MMs + 1 psum->SBUF eviction w/ bias) are spread one
      # per 4 steps of block nt, filling the PE/ACT idle in the step tail.
      # ================= Phase 3: LSTM recurrence (p2 fused in) ====
      # Per-step: 4 gate groups f -> i -> g -> o, descale+activation per
      # group; c-chain overlaps o MMs; tail = sigmoid(o) || tanh(c) -> h.
      # Tile nt+1's xg m-groups (4 MMs + biased eviction into an SBUF block
      # tile) are emitted one per 4 steps, so they execute in the PE/ACT
      # idle of the step tail. Tile 0 is a prologue.
      with nc.named_scope("p3_lstm"), \
           tc.tile_pool(name="xstr", bufs=2) as xstream, \
           tc.tile_pool(name="gsb", bufs=3) as gpool3, \
           tc.tile_pool(name="tmp3", bufs=3) as tpool, \
           tc.tile_pool(name="hrot", bufs=3) as hpool, \
           tc.tile_pool(name="whh", bufs=1) as whhpool, \
           tc.tile_pool(name="sqp", bufs=2) as sqpool, \
           tc.tile_pool(name="ps2", bufs=2, space="PSUM") as ps2pool, \
           tc.tile_pool(name="ps4a", bufs=1, space="PSUM") as ps4apool, \
           tc.tile_pool(name="ps4p", bufs=1, space="PSUM") as ps4ppool, \
           tc.tile_pool(name="ps3", bufs=1, space="PSUM") as ps3pool:
        if rec_repeat == 0:
            nc.vector.memset(hr_all, 0.0)
        else:
            whhT_sb = whhpool.tile([128, KC, G4],
                                   mybir.dt.float8e4 if whh_fp8 else BF16)
            nc.sync.dma_start(whhT_sb, whhT_d.rearrange("(k p) g -> p k g", p=128))

        def descale_add(out, ps, xg_slice):
            if whh_fp8:
                nc.vector.scalar_tensor_tensor(
                    out, ps, 0.125, xg_slice, op0=ALU.mult, op1=ALU.add)
            else:
                nc.vector.tensor_add(out, ps, xg_slice)

        def emit_p2_group(dst_blk, nt, m):
            ps = ps2pool.tile([128, TT], F32)
            for k in range(KC):
                nc.tensor.matmul(ps, wihT_sb[:, k, m * 128:(m + 1) * 128],
                                 eT_sb[:, k, nt * TT:(nt + 1) * TT],
                                 start=(k == 0), stop=(k == KC - 1))
            nc.scalar.activation(
                dst_blk[:, m, :, :].rearrange("p a b -> p (a b)"), ps, IDENT,
                bias=biasg_sb[:, m:m + 1], scale=1.0)

        # attention GEMM for a finished token tile, emitted inside the step
        # loop so the MMs/tanh run in the recurrence's idle windows
        def emit_attn_group(nt, mo, sq_list):
            ps = ps4apool.tile([128, TT], F32)
            for k in range(KC):
                nc.tensor.matmul(ps, wword_sb[:, k, mo * 128:(mo + 1) * 128],
                                 hr_all[:, k, nt * TT:(nt + 1) * TT],
                                 start=(k == 0), stop=(k == KC - 1))
            sq = sqpool.tile([128, TT], BF16, tag=f"sq{mo}")
            nc.scalar.activation(sq, ps, TANH, bias=bword_sb[:, mo:mo + 1],
                                 scale=1.0)
            sq_list.append(sq)

        def emit_attn_proj(nt, sq_list):
            ps2 = ps4ppool.tile([1, TT], F32)
            for mo in range(KC):
                nc.tensor.matmul(ps2, wproj_sb[:, mo, :], sq_list[mo],
                                 start=(mo == 0), stop=(mo == KC - 1))
            nc.vector.tensor_copy(scores_sb[0:1, nt * TT:(nt + 1) * TT], ps2)

        for rep in range(rec_repeat):
          if rep > 0:
            nc.vector.memset(cT, 0.0)
          h_prev = hzero
          with nc.named_scope("p2_xg"):
              cur_blk = xstream.tile([128, MC, SPT, BL], F32, tag="xgblk")
              for m in range(MC):
                  emit_p2_group(cur_blk, 0, m)
          nxt_blk = None
          attn_sq = []
          for s in range(S):
              nt = s // SPT
              sp = s % SPT
              if sp == 0:
                  if s > 0:
                      cur_blk = nxt_blk
                  if nt + 1 < NTT:
                      nxt_blk = xstream.tile([128, MC, SPT, BL], F32,
                                             tag="xgblk")
              if nt + 1 < NTT and sp % 4 == 0:
                  emit_p2_group(nxt_blk, nt + 1, sp // 4)
              if nt >= 1 and rep == rec_repeat - 1:
                  # attention for the previous (finished) token tile
                  if sp in (2, 10, 18, 26):
                      if sp == 2:
                          attn_sq = []
                      emit_attn_group(nt - 1, (sp - 2) // 8, attn_sq)
                  elif sp == 34:
                      emit_attn_proj(nt - 1, attn_sq)
              xg_t = cur_blk[:, :, s % SPT, :]
              # 4 gate groups, issued f -> i -> g -> o so each gate's
              # descale+activation starts right after its own 16 MMs; the
              # c-chain overlaps the o MMs and the tail is just
              # sigmoid(o) || tanh(c) -> h-mul.
              ps_f = ps3pool.tile([128, 4, BL], F32, tag="ps_f")
              ps_i = ps3pool.tile([128, 4, BL], F32, tag="ps_i")
              ps_g = ps3pool.tile([128, 4, BL], F32, tag="ps_g")
              ps_o = ps3pool.tile([128, 4, BL], F32, tag="ps_o")
              for ps, m0 in ((ps_f, 4), (ps_i, 0), (ps_g, 8), (ps_o, 12)):
                  for j in range(4):
                      m = m0 + j
                      for k in range(KC):
                          nc.tensor.matmul(ps[:, j, :],
                                           whhT_sb[:, k, m * 128:(m + 1) * 128],
                                           hzero[:, k, :] if free_run
                                           else h_prev[:, k, :],
                                           start=(k == 0), stop=(k == KC - 1))
              gf = gpool3.tile([128, 4, BL], F32, tag="gf")
              descale_add(gf, ps_f, xg_t[:, 4:8, :])
              nc.scalar.activation(gf, gf, SIG)
              gi = gpool3.tile([128, 4, BL], F32, tag="gi")
              descale_add(gi, ps_i, xg_t[:, 0:4, :])
              nc.scalar.activation(gi, gi, SIG)
              nc.vector.tensor_mul(cT, gf, cT)        # f*c during g/o MMs
              gg = gpool3.tile([128, 4, BL], F32, tag="gg")
              descale_add(gg, ps_g, xg_t[:, 8:12, :])
              nc.scalar.activation(gg, gg, TANH)
              go = gpool3.tile([128, 4, BL], F32, tag="go")
              descale_add(go, ps_o, xg_t[:, 12:16, :])
              ig = tpool.tile([128, 4, BL], F32, tag="ig")
              nc.vector.tensor_mul(ig, gi, gg)
              nc.vector.tensor_add(cT, cT, ig)
              nc.scalar.activation(go, go, SIG)       # queued before tanh_c
              th = tpool.tile([128, 4, BL], F32, tag="th")
              nc.scalar.activation(th, cT, TANH)
              h_t = hpool.tile([128, KC, BL], BF16, tag="h")
              nc.vector.tensor_mul(h_t, go, th)
              nc.vector.tensor_scalar_max(
                  hr_all[:, :, s * BL:(s + 1) * BL], h_t, 0.0)
              h_prev = h_t
          if rec_repeat > 0 and rep == rec_repeat - 1:
              # last token tile's attention (no following block to hide in)
              attn_sq = []
              for mo in range(KC):
                  emit_attn_group(NTT - 1, mo, attn_sq)
              emit_attn_proj(NTT - 1, attn_sq)

    # ================= Phase 4: softmax + ctx + FC ===============
    with nc.named_scope("p4_attn"), \
         tc.tile_pool(name="p4", bufs=4) as p4pool, \
         tc.tile_pool(name="wh", bufs=1) as whpool, \
         tc.tile_pool(name="ps4b", bufs=2, space="PSUM") as ps4bpool:
      for _p4rep in range(p4_repeat):
        # softmax over sequence, per batch element
        nc.sync.dma_start(scr_dram.rearrange("(o t) -> o t", o=1), scores_sb)
        sc_bs = p4pool.tile([BL, S], F32)
        nc.sync.dma_start(sc_bs, scr_dram.rearrange("(s b) -> b s", b=BL))
        mx = p4pool.tile([BL, 1], F32)
        nc.vector.tensor_reduce(mx, sc_bs, axis=AX_X, op=ALU.max)
        nc.vector.tensor_scalar_mul(mx, mx, -1.0)
        at = p4pool.tile([BL, S], F32)
        nc.scalar.activation(at, sc_bs, EXP, bias=mx[:, 0:1], scale=1.0)
        sm = p4pool.tile([BL, 1], F32)
        nc.vector.tensor_reduce(sm, at, axis=AX_X, op=ALU.add)
        nc.vector.reciprocal(sm, sm)
        nc.vector.tensor_scalar_mul(at, at, sm)
        nc.sync.dma_start(attn_dram.rearrange("(s b) -> b s", b=BL), at)
        attn_bc = whpool.tile([128, NTOK], F32, tag="abc")
        nc.sync.dma_start(
            attn_bc,
            bass.AP(tensor=attn_dram.tensor, offset=0, ap=[[0, 128], [1, NTOK]]))

        # ctx = sum_s attn * relu(h)
        for ch in range(KC):
            wh = whpool.tile([128, NTOK], F32, tag="wh")
            nc.vector.tensor_mul(wh, hr_all[:, ch, :], attn_bc)
            nc.vector.tensor_reduce(ctxT_sb[:, ch, :],
                                    wh.rearrange("p (s b) -> p b s", b=BL),
                                    axis=AX_X, op=ALU.add)

        # logits + softmax
        psL = ps4bpool.tile([BL, C], F32)
        for ch in range(KC):
            nc.tensor.matmul(psL, ctxT_sb[:, ch, :], fcwT_sb[:, ch, :],
                             start=(ch == 0), stop=(ch == KC - 1))
        lg = p4pool.tile([BL, C], F32)
        nc.vector.tensor_add(lg, psL, fcb_bc)
        mx2 = p4pool.tile([BL, 1], F32)
        nc.vector.tensor_reduce(mx2, lg, axis=AX_X, op=ALU.max)
        nc.vector.tensor_scalar_mul(mx2, mx2, -1.0)
        pe = p4pool.tile([BL, C], F32)
        nc.scalar.activation(pe, lg, EXP, bias=mx2[:, 0:1], scale=1.0)
        sm2 = p4pool.tile([BL, 1], F32)
        nc.vector.tensor_reduce(sm2, pe, axis=AX_X, op=ALU.add)
        nc.vector.reciprocal(sm2, sm2)
        nc.vector.tensor_scalar_mul(pe, pe, sm2)
        nc.sync.dma_start(out_d, pe)
    _stack.close()


def build_nc(S=S, V=V, **bkw):
    nc = bacc.Bacc("TRN2", target_bir_lowering=False, debug=False,
                   num_devices=NCORES)
    NTOK = S * BL
    whh_dt = mybir.dt.float8e4 if bkw.get("whh_fp8") else BF16
    io = {
        "idx": nc.dram_tensor("idx", [NTOK], I32, kind="ExternalInput").ap(),
        "embed": nc.dram_tensor("embed", [V, D], BF16, kind="ExternalInput").ap(),
        "wihT": nc.dram_tensor("wihT", [D, G4], BF16, kind="ExternalInput").ap(),
        "whhT": nc.dram_tensor("whhT", [Co, G4], whh_dt, kind="ExternalInput").ap(),
        "biasg": nc.dram_tensor("biasg", [G4], F32, kind="ExternalInput").ap(),
        "wword": nc.dram_tensor("wword", [Co, Co], BF16, kind="ExternalInput").ap(),
        "bword": nc.dram_tensor("bword", [Co], F32, kind="ExternalInput").ap(),
        "wproj": nc.dram_tensor("wproj", [Co, 1], BF16, kind="ExternalInput").ap(),
        "fcwT": nc.dram_tensor("fcwT", [Co, C], F32, kind="ExternalInput").ap(),
        "fcb": nc.dram_tensor("fcb", [C], F32, kind="ExternalInput").ap(),
        "probs": nc.dram_tensor("probs", [BL, C], F32, kind="ExternalOutput").ap(),
    }
    with tile.TileContext(nc) as tc:
        build_body(tc, io, S=S, V=V, **bkw)
    nc.compile()
    return nc


def host_prep(inputs, whh_fp8=False):
    """Cast/transpose parameters on host; build per-core in_maps."""
    bf = ml_dtypes.bfloat16
    x = np.asarray(inputs["x"])
    common = {
        "embed": np.ascontiguousarray(np.asarray(inputs["embed"]).astype(bf)),
        "wihT": np.ascontiguousarray(np.asarray(inputs["W_ih"]).T.astype(bf)),
        "whhT": (np.ascontiguousarray((np.asarray(inputs["W_hh"]).T * 8.0).astype(ml_dtypes.float8_e4m3fn))
                  if whh_fp8 else
                  np.ascontiguousarray(np.asarray(inputs["W_hh"]).T.astype(bf))),
        "biasg": np.ascontiguousarray(
            (np.asarray(inputs["b_ih"]) + np.asarray(inputs["b_hh"])).astype(np.float32)),
        "wword": np.ascontiguousarray(np.asarray(inputs["weight_word"]).astype(bf)),
        "bword": np.ascontiguousarray(np.asarray(inputs["bias_word"])[:, 0].astype(np.float32)),
        "wproj": np.ascontiguousarray(np.asarray(inputs["weight_proj_word"]).astype(bf)),
        "fcwT": np.ascontiguousarray(np.asarray(inputs["fc_w"]).T.astype(np.float32)),
        "fcb": np.ascontiguousarray(np.asarray(inputs["fc_b"]).astype(np.float32)),
    }
    in_maps = []
    for c in range(NCORES):
        shard = x[c * BL:(c + 1) * BL, :]          # [BL, S]
        idx = np.ascontiguousarray(shard.T.reshape(-1).astype(np.int32))  # s-major
        in_maps.append({"idx": idx, **common})
    return in_maps


_NC_CACHE = {}
WHH_FP8 = True


def _get_nc():
    if "nc" not in _NC_CACHE:
        _NC_CACHE["nc"] = build_nc(whh_fp8=WHH_FP8)
    return _NC_CACHE["nc"]


def kernel(**inputs):
    nc = _get_nc()
    in_maps = host_prep(inputs, whh_fp8=WHH_FP8)
    res = run_bass_kernel_spmd(nc, in_maps, core_ids=list(range(NCORES)))
    probs = np.concatenate([res.results[c]["probs"] for c in range(NCORES)], axis=0)
    return probs.astype(np.float32)


def run_traced(inputs):
    """Like kernel() but with NTFF tracing; returns (probs, BassKernelResults)."""
    nc = _get_nc()
    in_maps = host_prep(inputs, whh_fp8=WHH_FP8)
    res = run_bass_kernel_spmd(nc, in_maps, core_ids=list(range(NCORES)),
                               trace=True)
    probs = np.concatenate([res.results[c]["probs"] for c in range(NCORES)], axis=0)
    return probs.astype(np.float32), res



# revision 37
# speedup vs baseline: 1.1109x; 1.0433x over previous
"""Trainium2 Bass kernel for CNN_Text-style LSTM classifier.

Model: embedding lookup -> 512-step LSTM -> attention pooling -> FC -> softmax.
Strategy: data-parallel over batch (B=64 -> 8 cores x 8). All parameters
replicated. Per core, tokens are ordered seq-major: t = s*BL + b.

kernel(**inputs) takes FULL numpy inputs (as produced by setup_inputs) and
returns the FULL [64, 10] float32 output.
"""
import numpy as np
import ml_dtypes

import concourse.bass as bass
import concourse.tile as tile
from concourse import bacc, masks, mybir
from concourse.bass_utils import run_bass_kernel_spmd

BF16 = mybir.dt.bfloat16
F32 = mybir.dt.float32
I32 = mybir.dt.int32

# Full-problem constants
V, D, Co, C = 50000, 512, 512, 10
B, S = 64, 512
NCORES = 8
BL = B // NCORES          # local batch per core
G4 = 4 * Co               # 2048 gate dim
KC = D // 128             # 4 contraction chunks (D == Co == 512)
MC = G4 // 128            # 16 gate-dim chunks

SIG = mybir.ActivationFunctionType.Sigmoid
TANH = mybir.ActivationFunctionType.Tanh
EXP = mybir.ActivationFunctionType.Exp
IDENT = mybir.ActivationFunctionType.Identity
AX_X = mybir.AxisListType.X
ALU = mybir.AluOpType


def build_body(tc, io, S=S, V=V, rec_repeat=1, g_repeat=1, p2_repeat=1, p4_repeat=1, whh_fp8=False, split_o=True, free_run=False):
    """Emit the whole per-core program. io: dict of dram APs."""
    nc = tc.nc
    NTOK = S * BL
    NROWT = NTOK // 128      # gather row-tiles
    TT = min(512, NTOK)      # token tile for phase2/4 GEMMs
    NTT = NTOK // TT         # number of token tiles
    SPT = TT // BL           # steps per token tile

    idx_d = io["idx"]; embed_d = io["embed"]
    wihT_d = io["wihT"]; whhT_d = io["whhT"]; biasg_d = io["biasg"]
    wword_d = io["wword"]; bword_d = io["bword"]; wproj_d = io["wproj"]
    fcwT_d = io["fcwT"]; fcb_d = io["fcb"]; out_d = io["probs"]

    NTT0 = NTOK // min(512, NTOK)
    scr_dram = nc.dram_tensor("sc_scr", [NTOK], F32, kind="Internal").ap()
    attn_dram = nc.dram_tensor("at_scr", [NTOK], F32, kind="Internal").ap()

    from contextlib import ExitStack
    _stack = ExitStack()
    const = _stack.enter_context(tc.tile_pool(name="const", bufs=1))
    state = _stack.enter_context(tc.tile_pool(name="state", bufs=1))

    # ---- constants to SBUF ----
    biasg_sb = const.tile([128, MC], F32)
    nc.sync.dma_start(biasg_sb, biasg_d.rearrange("(m p) -> p m", p=128))
    wword_sb = const.tile([128, KC, Co], BF16)
    nc.sync.dma_start(wword_sb, wword_d.rearrange("(k p) j -> p k j", p=128))
    bword_sb = const.tile([128, KC], F32)
    nc.sync.dma_start(bword_sb, bword_d.rearrange("(m p) -> p m", p=128))
    wproj_sb = const.tile([128, KC, 1], BF16)
    nc.sync.dma_start(wproj_sb, wproj_d.rearrange("(m p) o -> p m o", p=128))
    fcwT_sb = const.tile([128, KC, C], F32)
    nc.sync.dma_start(fcwT_sb, fcwT_d.rearrange("(k p) c -> p k c", p=128))
    fcb_bc = const.tile([BL, C], F32)
    nc.sync.dma_start(
        fcb_bc, bass.AP(tensor=fcb_d.tensor, offset=0, ap=[[0, BL], [1, C]]))
    idx_sb = const.tile([128, NROWT], I32)
    nc.sync.dma_start(idx_sb, idx_d.rearrange("(j p) -> p j", p=128))
    hzero = const.tile([128, KC, BL], BF16)
    nc.vector.memset(hzero, 0.0)
    ident = const.tile([128, 128], BF16)
    masks.make_identity(nc, ident[:])
    # scaled f32 identity: streams xg into the gate PSUM accumulation
    # pre-scaled to match the x8 fp8 weight scale (activation applies 0.125)
    identg = const.tile([128, 128], F32)
    masks.make_identity(nc, identg[:])
    if whh_fp8:
        nc.vector.tensor_scalar_mul(identg, identg, 8.0)
    wihT_sb = const.tile([128, KC, G4], BF16)
    nc.sync.dma_start(wihT_sb, wihT_d.rearrange("(k p) g -> p k g", p=128))

    # ---- persistent state ----
    hr_all = state.tile([128, KC, NTOK], BF16)   # relu(h), transposed layout
    cT = state.tile([128, KC, BL], F32)
    nc.vector.memset(cT, 0.0)
    scores_sb = state.tile([1, NTOK], F32)
    ctxT_sb = state.tile([128, KC, BL], F32)

    # ================= Phase 1: gather + on-chip transpose =======
    # Gather 128-token row tiles, PE-transpose each 128x128 block, keep
    # e^T resident in SBUF (no DRAM roundtrip, no transpose-DMA).
    with tc.tile_pool(name="etsb", bufs=1) as etpool:
      eT_sb = etpool.tile([128, KC, NTOK], BF16)
      with nc.named_scope("p1_gather"), \
           tc.tile_pool(name="gat", bufs=4) as gpool, \
           tc.tile_pool(name="ps1", bufs=4, space="PSUM") as ps1pool:
        for _grep in range(g_repeat):
          for j in range(NROWT):
            g_sb = gpool.tile([128, D], BF16)
            nc.gpsimd.indirect_dma_start(
                out=g_sb[:], out_offset=None, in_=embed_d[:],
                in_offset=bass.IndirectOffsetOnAxis(ap=idx_sb[:, j:j + 1], axis=0))
            for k in range(KC):
                pt = ps1pool.tile([128, 128], BF16)
                nc.tensor.transpose(pt, g_sb[:, k * 128:(k + 1) * 128], ident)
                nc.vector.tensor_copy(eT_sb[:, k, j * 128:(j + 1) * 128], pt)

      # ================= Phase 2+3 fused ===========================
      # xg for token-tile nt lives in an SBUF block tile [128, MC, SPT, BL]
      # (no DRAM roundtrip). Tile 0 is produced in a prologue; tile nt+1's
      # 16 (m)-groups (4 MMs + 1 psum->SBUF eviction w/ bias) are spread one
      # per 4 steps of block nt, filling the PE/ACT idle in the step tail.
      # ================= Phase 3: LSTM recurrence (p2 fused in) ====
      # Per-step: 4 gate groups f -> i -> g -> o, descale+activation per
      # group; c-chain overlaps o MMs; tail = sigmoid(o) || tanh(c) -> h.
      # Tile nt+1's xg m-groups (4 MMs + biased eviction into an SBUF block
      # tile) are emitted one per 4 steps, so they execute in the PE/ACT
      # idle of the step tail. Tile 0 is a prologue.
      with nc.named_scope("p3_lstm"), \
           tc.tile_pool(name="xstr", bufs=2) as xstream, \
           tc.tile_pool(name="gsb", bufs=3) as gpool3, \
           tc.tile_pool(name="tmp3", bufs=3) as tpool, \
           tc.tile_pool(name="hrot", bufs=3) as hpool, \
           tc.tile_pool(name="whh", bufs=1) as whhpool, \
           tc.tile_pool(name="sqp", bufs=2) as sqpool, \
           tc.tile_pool(name="ps2", bufs=2, space="PSUM") as ps2pool, \
           tc.tile_pool(name="ps4a", bufs=1, space="PSUM") as ps4apool, \
           tc.tile_pool(name="ps4p", bufs=1, space="PSUM") as ps4ppool, \
           tc.tile_pool(name="ps3", bufs=1, space="PSUM") as ps3pool:
        if rec_repeat == 0:
            nc.vector.memset(hr_all, 0.0)
        else:
            whhT_sb = whhpool.tile([128, KC, G4],
                                   mybir.dt.float8e4 if whh_fp8 else BF16)
            nc.sync.dma_start(whhT_sb, whhT_d.rearrange("(k p) g -> p k g", p=128))

        def descale_add(out, ps, xg_slice):
            if whh_fp8:
                nc.vector.scalar_tensor_tensor(
                    out, ps, 0.125, xg_slice, op0=ALU.mult, op1=ALU.add)
            else:
                nc.vector.tensor_add(out, ps, xg_slice)

        def emit_p2_group(dst_blk, nt, m):
            ps = ps2pool.tile([128, TT], F32)
            for k in range(KC):
                nc.tensor.matmul(ps, wihT_sb[:, k, m * 128:(m + 1) * 128],
                                 eT_sb[:, k, nt * TT:(nt + 1) * TT],
                                 start=(k == 0), stop=(k == KC - 1))
            nc.scalar.activation(
                dst_blk[:, m, :, :].rearrange("p a b -> p (a b)"), ps, IDENT,
                bias=biasg_sb[:, m:m + 1], scale=1.0)

        # attention GEMM for a finished token tile, emitted inside the step
        # loop so the MMs/tanh run in the recurrence's idle windows
        def emit_attn_group(nt, mo, sq_list):
            ps = ps4apool.tile([128, TT], F32)
            for k in range(KC):
                nc.tensor.matmul(ps, wword_sb[:, k, mo * 128:(mo + 1) * 128],
                                 hr_all[:, k, nt * TT:(nt + 1) * TT],
                                 start=(k == 0), stop=(k == KC - 1))
            sq = sqpool.tile([128, TT], BF16, tag=f"sq{mo}")
            nc.scalar.activation(sq, ps, TANH, bias=bword_sb[:, mo:mo + 1],
                                 scale=1.0)
            sq_list.append(sq)

        def emit_attn_proj(nt, sq_list):
            ps2 = ps4ppool.tile([1, TT], F32)
            for mo in range(KC):
                nc.tensor.matmul(ps2, wproj_sb[:, mo, :], sq_list[mo],
                                 start=(mo == 0), stop=(mo == KC - 1))
            nc.vector.tensor_copy(scores_sb[0:1, nt * TT:(nt + 1) * TT], ps2)

        for rep in range(rec_repeat):
          if rep > 0:
            nc.vector.memset(cT, 0.0)
          h_prev = hzero
          with nc.named_scope("p2_xg"):
              cur_blk = xstream.tile([128, MC, SPT, BL], F32, tag="xgblk")
              for m in range(MC):
                  emit_p2_group(cur_blk, 0, m)
          nxt_blk = None
          attn_sq = []
          for s in range(S):
              nt = s // SPT
              sp = s % SPT
              if sp == 0:
                  if s > 0:
                      cur_blk = nxt_blk
                  if nt + 1 < NTT:
                      nxt_blk = xstream.tile([128, MC, SPT, BL], F32,
                                             tag="xgblk")
              if nt + 1 < NTT and sp % 4 == 0:
                  emit_p2_group(nxt_blk, nt + 1, sp // 4)
              if nt >= 1 and rep == rec_repeat - 1:
                  # attention for the previous (finished) token tile
                  if sp in (2, 10, 18, 26):
                      if sp == 2:
                          attn_sq = []
                      emit_attn_group(nt - 1, (sp - 2) // 8, attn_sq)
                  elif sp == 34:
                      emit_attn_proj(nt - 1, attn_sq)
              xg_t = cur_blk[:, :, s % SPT, :]
              # 4 gate groups, issued f -> i -> g -> o so each gate's
              # descale+activation starts right after its own 16 MMs; the
              # c-chain overlaps the o MMs and the tail is just
              # sigmoid(o) || tanh(c) -> h-mul.
              ps_f = ps3pool.tile([128, 4, BL], F32, tag="ps_f")
              ps_i = ps3pool.tile([128, 4, BL], F32, tag="ps_i")
              ps_g = ps3pool.tile([128, 4, BL], F32, tag="ps_g")
              ps_o = ps3pool.tile([128, 4, BL], F32, tag="ps_o")
              # xg enters each gate's PSUM bank via ONE identity-stationary
              # MM covering the whole [128, 4, BL] region (a single
              # start=True per bank, so no sibling-region has_written
              # corruption); W.h MMs accumulate on top; activations read
              # PSUM directly with the 0.125 descale in the scale.
              DSC = 0.125 if whh_fp8 else 1.0
              for ps, m0 in ((ps_f, 4), (ps_i, 0), (ps_g, 8), (ps_o, 12)):
                  nc.tensor.matmul(ps[:, :, :], identg,
                                   xg_t[:, m0:m0 + 4, :],
                                   start=True, stop=False)
              for ps, m0 in ((ps_f, 4), (ps_i, 0), (ps_g, 8), (ps_o, 12)):
                  for j in range(4):
                      m = m0 + j
                      for k in range(KC):
                          nc.tensor.matmul(ps[:, j, :],
                                           whhT_sb[:, k, m * 128:(m + 1) * 128],
                                           hzero[:, k, :] if free_run
                                           else h_prev[:, k, :],
                                           start=False, stop=(k == KC - 1))
              gf = gpool3.tile([128, 4, BL], F32, tag="gf")
              nc.scalar.activation(gf, ps_f, SIG, scale=DSC)
              gi = gpool3.tile([128, 4, BL], F32, tag="gi")
              nc.scalar.activation(gi, ps_i, SIG, scale=DSC)
              nc.vector.tensor_mul(cT, gf, cT)        # f*c during g/o MMs
              gg = gpool3.tile([128, 4, BL], F32, tag="gg")
              nc.scalar.activation(gg, ps_g, TANH, scale=DSC)
              go = gpool3.tile([128, 4, BL], F32, tag="go")
              ig = tpool.tile([128, 4, BL], F32, tag="ig")
              nc.vector.tensor_mul(ig, gi, gg)
              nc.vector.tensor_add(cT, cT, ig)
              nc.scalar.activation(go, ps_o, SIG, scale=DSC)  # before tanh_c
              th = tpool.tile([128, 4, BL], F32, tag="th")
              nc.scalar.activation(th, cT, TANH)
              h_t = hpool.tile([128, KC, BL], BF16, tag="h")
              nc.vector.tensor_mul(h_t, go, th)
              nc.vector.tensor_scalar_max(
                  hr_all[:, :, s * BL:(s + 1) * BL], h_t, 0.0)
              h_prev = h_t
          if rec_repeat > 0 and rep == rec_repeat - 1:
              # last token tile's attention (no following block to hide in)
              attn_sq = []
              for mo in range(KC):
                  emit_attn_group(NTT - 1, mo, attn_sq)
              emit_attn_proj(NTT - 1, attn_sq)

    # ================= Phase 4: softmax + ctx + FC ===============
    with nc.named_scope("p4_attn"), \
         tc.tile_pool(name="p4", bufs=4) as p4pool, \
         tc.tile_pool(name="wh", bufs=1) as whpool, \
         tc.tile_pool(name="ps4b", bufs=2, space="PSUM") as ps4bpool:
      for _p4rep in range(p4_repeat):
        # softmax over sequence, per batch element
        nc.sync.dma_start(scr_dram.rearrange("(o t) -> o t", o=1), scores_sb)
        sc_bs = p4pool.tile([BL, S], F32)
        nc.sync.dma_start(sc_bs, scr_dram.rearrange("(s b) -> b s", b=BL))
        mx = p4pool.tile([BL, 1], F32)
        nc.vector.tensor_reduce(mx, sc_bs, axis=AX_X, op=ALU.max)
        nc.vector.tensor_scalar_mul(mx, mx, -1.0)
        at = p4pool.tile([BL, S], F32)
        nc.scalar.activation(at, sc_bs, EXP, bias=mx[:, 0:1], scale=1.0)
        sm = p4pool.tile([BL, 1], F32)
        nc.vector.tensor_reduce(sm, at, axis=AX_X, op=ALU.add)
        nc.vector.reciprocal(sm, sm)
        nc.vector.tensor_scalar_mul(at, at, sm)
        nc.sync.dma_start(attn_dram.rearrange("(s b) -> b s", b=BL), at)
        attn_bc = whpool.tile([128, NTOK], F32, tag="abc")
        nc.sync.dma_start(
            attn_bc,
            bass.AP(tensor=attn_dram.tensor, offset=0, ap=[[0, 128], [1, NTOK]]))

        # ctx = sum_s attn * relu(h)
        for ch in range(KC):
            wh = whpool.tile([128, NTOK], F32, tag="wh")
            nc.vector.tensor_mul(wh, hr_all[:, ch, :], attn_bc)
            nc.vector.tensor_reduce(ctxT_sb[:, ch, :],
                                    wh.rearrange("p (s b) -> p b s", b=BL),
                                    axis=AX_X, op=ALU.add)

        # logits + softmax
        psL = ps4bpool.tile([BL, C], F32)
        for ch in range(KC):
            nc.tensor.matmul(psL, ctxT_sb[:, ch, :], fcwT_sb[:, ch, :],
                             start=(ch == 0), stop=(ch == KC - 1))
        lg = p4pool.tile([BL, C], F32)
        nc.vector.tensor_add(lg, psL, fcb_bc)
        mx2 = p4pool.tile([BL, 1], F32)
        nc.vector.tensor_reduce(mx2, lg, axis=AX_X, op=ALU.max)
        nc.vector.tensor_scalar_mul(mx2, mx2, -1.0)
        pe = p4pool.tile([BL, C], F32)
        nc.scalar.activation(pe, lg, EXP, bias=mx2[:, 0:1], scale=1.0)
        sm2 = p4pool.tile([BL, 1], F32)
        nc.vector.tensor_reduce(sm2, pe, axis=AX_X, op=ALU.add)
        nc.vector.reciprocal(sm2, sm2)
        nc.vector.tensor_scalar_mul(pe, pe, sm2)
        nc.sync.dma_start(out_d, pe)
    _stack.close()


def build_nc(S=S, V=V, **bkw):
    nc = bacc.Bacc("TRN2", target_bir_lowering=False, debug=False,
                   num_devices=NCORES)
    NTOK = S * BL
    whh_dt = mybir.dt.float8e4 if bkw.get("whh_fp8") else BF16
    io = {
        "idx": nc.dram_tensor("idx", [NTOK], I32, kind="ExternalInput").ap(),
        "embed": nc.dram_tensor("embed", [V, D], BF16, kind="ExternalInput").ap(),
        "wihT": nc.dram_tensor("wihT", [D, G4], BF16, kind="ExternalInput").ap(),
        "whhT": nc.dram_tensor("whhT", [Co, G4], whh_dt, kind="ExternalInput").ap(),
        "biasg": nc.dram_tensor("biasg", [G4], F32, kind="ExternalInput").ap(),
        "wword": nc.dram_tensor("wword", [Co, Co], BF16, kind="ExternalInput").ap(),
        "bword": nc.dram_tensor("bword", [Co], F32, kind="ExternalInput").ap(),
        "wproj": nc.dram_tensor("wproj", [Co, 1], BF16, kind="ExternalInput").ap(),
        "fcwT": nc.dram_tensor("fcwT", [Co, C], F32, kind="ExternalInput").ap(),
        "fcb": nc.dram_tensor("fcb", [C], F32, kind="ExternalInput").ap(),
        "probs": nc.dram_tensor("probs", [BL, C], F32, kind="ExternalOutput").ap(),
    }
    with tile.TileContext(nc) as tc:
        build_body(tc, io, S=S, V=V, **bkw)
    nc.compile()
    return nc


def host_prep(inputs, whh_fp8=False):
    """Cast/transpose parameters on host; build per-core in_maps."""
    bf = ml_dtypes.bfloat16
    x = np.asarray(inputs["x"])
    common = {
        "embed": np.ascontiguousarray(np.asarray(inputs["embed"]).astype(bf)),
        "wihT": np.ascontiguousarray(np.asarray(inputs["W_ih"]).T.astype(bf)),
        "whhT": (np.ascontiguousarray((np.asarray(inputs["W_hh"]).T * 8.0).astype(ml_dtypes.float8_e4m3fn))
                  if whh_fp8 else
                  np.ascontiguousarray(np.asarray(inputs["W_hh"]).T.astype(bf))),
        "biasg": np.ascontiguousarray(
            (np.asarray(inputs["b_ih"]) + np.asarray(inputs["b_hh"])).astype(np.float32)),
        "wword": np.ascontiguousarray(np.asarray(inputs["weight_word"]).astype(bf)),
        "bword": np.ascontiguousarray(np.asarray(inputs["bias_word"])[:, 0].astype(np.float32)),
        "wproj": np.ascontiguousarray(np.asarray(inputs["weight_proj_word"]).astype(bf)),
        "fcwT": np.ascontiguousarray(np.asarray(inputs["fc_w"]).T.astype(np.float32)),
        "fcb": np.ascontiguousarray(np.asarray(inputs["fc_b"]).astype(np.float32)),
    }
    in_maps = []
    for c in range(NCORES):
        shard = x[c * BL:(c + 1) * BL, :]          # [BL, S]
        idx = np.ascontiguousarray(shard.T.reshape(-1).astype(np.int32))  # s-major
        in_maps.append({"idx": idx, **common})
    return in_maps


_NC_CACHE = {}
WHH_FP8 = True


def _get_nc():
    if "nc" not in _NC_CACHE:
        _NC_CACHE["nc"] = build_nc(whh_fp8=WHH_FP8)
    return _NC_CACHE["nc"]


def kernel(**inputs):
    nc = _get_nc()
    in_maps = host_prep(inputs, whh_fp8=WHH_FP8)
    res = run_bass_kernel_spmd(nc, in_maps, core_ids=list(range(NCORES)))
    probs = np.concatenate([res.results[c]["probs"] for c in range(NCORES)], axis=0)
    return probs.astype(np.float32)


def run_traced(inputs):
    """Like kernel() but with NTFF tracing; returns (probs, BassKernelResults)."""
    nc = _get_nc()
    in_maps = host_prep(inputs, whh_fp8=WHH_FP8)
    res = run_bass_kernel_spmd(nc, in_maps, core_ids=list(range(NCORES)),
                               trace=True)
    probs = np.concatenate([res.results[c]["probs"] for c in range(NCORES)], axis=0)
    return probs.astype(np.float32), res



# revision 40
# speedup vs baseline: 1.1667x; 1.0502x over previous
"""Trainium2 Bass kernel for CNN_Text-style LSTM classifier.

Model: embedding lookup -> 512-step LSTM -> attention pooling -> FC -> softmax.
Strategy: data-parallel over batch (B=64 -> 8 cores x 8). All parameters
replicated. Per core, tokens are ordered seq-major: t = s*BL + b.

kernel(**inputs) takes FULL numpy inputs (as produced by setup_inputs) and
returns the FULL [64, 10] float32 output.
"""
import numpy as np
import ml_dtypes

import concourse.bass as bass
import concourse.tile as tile
from concourse import bacc, masks, mybir
from concourse.bass_utils import run_bass_kernel_spmd

BF16 = mybir.dt.bfloat16
F32 = mybir.dt.float32
I32 = mybir.dt.int32

# Full-problem constants
V, D, Co, C = 50000, 512, 512, 10
B, S = 64, 512
NCORES = 8
BL = B // NCORES          # local batch per core
G4 = 4 * Co               # 2048 gate dim
KC = D // 128             # 4 contraction chunks (D == Co == 512)
MC = G4 // 128            # 16 gate-dim chunks

SIG = mybir.ActivationFunctionType.Sigmoid
TANH = mybir.ActivationFunctionType.Tanh
EXP = mybir.ActivationFunctionType.Exp
IDENT = mybir.ActivationFunctionType.Identity
AX_X = mybir.AxisListType.X
ALU = mybir.AluOpType


def build_body(tc, io, S=S, V=V, rec_repeat=1, g_repeat=1, p2_repeat=1, p4_repeat=1, whh_fp8=False, split_o=True, free_run=False):
    """Emit the whole per-core program. io: dict of dram APs."""
    nc = tc.nc
    NTOK = S * BL
    NROWT = NTOK // 128      # gather row-tiles
    TT = min(512, NTOK)      # token tile for phase2/4 GEMMs
    NTT = NTOK // TT         # number of token tiles
    SPT = TT // BL           # steps per token tile

    idx_d = io["idx"]; embed_d = io["embed"]
    wihT_d = io["wihT"]; whhT_d = io["whhT"]; biasg_d = io["biasg"]
    wword_d = io["wword"]; bword_d = io["bword"]; wproj_d = io["wproj"]
    fcwT_d = io["fcwT"]; fcb_d = io["fcb"]; out_d = io["probs"]

    NTT0 = NTOK // min(512, NTOK)
    scr_dram = nc.dram_tensor("sc_scr", [NTOK], F32, kind="Internal").ap()
    attn_dram = nc.dram_tensor("at_scr", [NTOK], F32, kind="Internal").ap()

    from contextlib import ExitStack
    _stack = ExitStack()
    const = _stack.enter_context(tc.tile_pool(name="const", bufs=1))
    state = _stack.enter_context(tc.tile_pool(name="state", bufs=1))

    # ---- constants to SBUF ----
    biasg_sb = const.tile([128, MC], F32)
    nc.sync.dma_start(biasg_sb, biasg_d.rearrange("(m p) -> p m", p=128))
    wword_sb = const.tile([128, KC, Co], BF16)
    nc.sync.dma_start(wword_sb, wword_d.rearrange("(k p) j -> p k j", p=128))
    bword_sb = const.tile([128, KC], F32)
    nc.sync.dma_start(bword_sb, bword_d.rearrange("(m p) -> p m", p=128))
    wproj_sb = const.tile([128, KC, 1], BF16)
    nc.sync.dma_start(wproj_sb, wproj_d.rearrange("(m p) o -> p m o", p=128))
    fcwT_sb = const.tile([128, KC, C], F32)
    nc.sync.dma_start(fcwT_sb, fcwT_d.rearrange("(k p) c -> p k c", p=128))
    fcb_bc = const.tile([BL, C], F32)
    nc.sync.dma_start(
        fcb_bc, bass.AP(tensor=fcb_d.tensor, offset=0, ap=[[0, BL], [1, C]]))
    idx_sb = const.tile([128, NROWT], I32)
    nc.sync.dma_start(idx_sb, idx_d.rearrange("(j p) -> p j", p=128))
    hzero = const.tile([128, KC, BL], BF16)
    nc.vector.memset(hzero, 0.0)
    ident = const.tile([128, 128], BF16)
    masks.make_identity(nc, ident[:])
    # scaled f32 identity: streams xg into the gate PSUM accumulation
    # pre-scaled to match the x8 fp8 weight scale (activation applies 0.125)
    identg = const.tile([128, 128], BF16)
    masks.make_identity(nc, identg[:])
    if whh_fp8:
        nc.vector.tensor_scalar_mul(identg, identg, 8.0)
    wihT_sb = const.tile([128, KC, G4], BF16)
    nc.sync.dma_start(wihT_sb, wihT_d.rearrange("(k p) g -> p k g", p=128))

    # ---- persistent state ----
    hr_all = state.tile([128, KC, NTOK], BF16)   # relu(h), transposed layout
    cT = state.tile([128, KC, BL], F32)
    nc.vector.memset(cT, 0.0)
    scores_sb = state.tile([1, NTOK], F32)
    ctxT_sb = state.tile([128, KC, BL], F32)

    # ================= Phase 1: gather + on-chip transpose =======
    # Gather 128-token row tiles, PE-transpose each 128x128 block, keep
    # e^T resident in SBUF (no DRAM roundtrip, no transpose-DMA).
    with tc.tile_pool(name="etsb", bufs=1) as etpool:
      eT_sb = etpool.tile([128, KC, NTOK], BF16)
      with nc.named_scope("p1_gather"), \
           tc.tile_pool(name="gat", bufs=4) as gpool, \
           tc.tile_pool(name="ps1", bufs=4, space="PSUM") as ps1pool:
        for _grep in range(g_repeat):
          for j in range(NROWT):
            g_sb = gpool.tile([128, D], BF16)
            nc.gpsimd.indirect_dma_start(
                out=g_sb[:], out_offset=None, in_=embed_d[:],
                in_offset=bass.IndirectOffsetOnAxis(ap=idx_sb[:, j:j + 1], axis=0))
            for k in range(KC):
                pt = ps1pool.tile([128, 128], BF16)
                nc.tensor.transpose(pt, g_sb[:, k * 128:(k + 1) * 128], ident)
                nc.vector.tensor_copy(eT_sb[:, k, j * 128:(j + 1) * 128], pt)

      # ================= Phase 2+3 fused ===========================
      # xg for token-tile nt lives in an SBUF block tile [128, MC, SPT, BL]
      # (no DRAM roundtrip). Tile 0 is produced in a prologue; tile nt+1's
      # 16 (m)-groups (4 MMs + 1 psum->SBUF eviction w/ bias) are spread one
      # per 4 steps of block nt, filling the PE/ACT idle in the step tail.
      # ================= Phase 3: LSTM recurrence (p2 fused in) ====
      # Per-step: 4 gate groups f -> i -> g -> o, descale+activation per
      # group; c-chain overlaps o MMs; tail = sigmoid(o) || tanh(c) -> h.
      # Tile nt+1's xg m-groups (4 MMs + biased eviction into an SBUF block
      # tile) are emitted one per 4 steps, so they execute in the PE/ACT
      # idle of the step tail. Tile 0 is a prologue.
      with nc.named_scope("p3_lstm"), \
           tc.tile_pool(name="xstr", bufs=2) as xstream, \
           tc.tile_pool(name="gsb", bufs=3) as gpool3, \
           tc.tile_pool(name="tmp3", bufs=3) as tpool, \
           tc.tile_pool(name="hrot", bufs=3) as hpool, \
           tc.tile_pool(name="whh", bufs=1) as whhpool, \
           tc.tile_pool(name="sqp", bufs=2) as sqpool, \
           tc.tile_pool(name="ps2", bufs=2, space="PSUM") as ps2pool, \
           tc.tile_pool(name="ps4a", bufs=1, space="PSUM") as ps4apool, \
           tc.tile_pool(name="ps4p", bufs=1, space="PSUM") as ps4ppool, \
           tc.tile_pool(name="ps3", bufs=1, space="PSUM") as ps3pool:
        if rec_repeat == 0:
            nc.vector.memset(hr_all, 0.0)
        else:
            whhT_sb = whhpool.tile([128, KC, G4],
                                   mybir.dt.float8e4 if whh_fp8 else BF16)
            nc.sync.dma_start(whhT_sb, whhT_d.rearrange("(k p) g -> p k g", p=128))

        def descale_add(out, ps, xg_slice):
            if whh_fp8:
                nc.vector.scalar_tensor_tensor(
                    out, ps, 0.125, xg_slice, op0=ALU.mult, op1=ALU.add)
            else:
                nc.vector.tensor_add(out, ps, xg_slice)

        def emit_p2_group(dst_blk, nt, m):
            ps = ps2pool.tile([128, TT], F32)
            for k in range(KC):
                nc.tensor.matmul(ps, wihT_sb[:, k, m * 128:(m + 1) * 128],
                                 eT_sb[:, k, nt * TT:(nt + 1) * TT],
                                 start=(k == 0), stop=(k == KC - 1))
            nc.scalar.activation(
                dst_blk[:, m, :, :].rearrange("p a b -> p (a b)"), ps, IDENT,
                bias=biasg_sb[:, m:m + 1], scale=1.0)

        # attention GEMM for a finished token tile, emitted inside the step
        # loop so the MMs/tanh run in the recurrence's idle windows
        def emit_attn_group(nt, mo, sq_list):
            ps = ps4apool.tile([128, TT], F32)
            for k in range(KC):
                nc.tensor.matmul(ps, wword_sb[:, k, mo * 128:(mo + 1) * 128],
                                 hr_all[:, k, nt * TT:(nt + 1) * TT],
                                 start=(k == 0), stop=(k == KC - 1))
            sq = sqpool.tile([128, TT], BF16, tag=f"sq{mo}")
            nc.scalar.activation(sq, ps, TANH, bias=bword_sb[:, mo:mo + 1],
                                 scale=1.0)
            sq_list.append(sq)

        def emit_attn_proj(nt, sq_list):
            ps2 = ps4ppool.tile([1, TT], F32)
            for mo in range(KC):
                nc.tensor.matmul(ps2, wproj_sb[:, mo, :], sq_list[mo],
                                 start=(mo == 0), stop=(mo == KC - 1))
            nc.vector.tensor_copy(scores_sb[0:1, nt * TT:(nt + 1) * TT], ps2)

        for rep in range(rec_repeat):
          if rep > 0:
            nc.vector.memset(cT, 0.0)
          h_prev = hzero
          with nc.named_scope("p2_xg"):
              cur_blk = xstream.tile([128, MC, SPT, BL], BF16, tag="xgblk")
              for m in range(MC):
                  emit_p2_group(cur_blk, 0, m)
          nxt_blk = None
          attn_sq = []
          for s in range(S):
              nt = s // SPT
              sp = s % SPT
              if sp == 0:
                  if s > 0:
                      cur_blk = nxt_blk
                  if nt + 1 < NTT:
                      nxt_blk = xstream.tile([128, MC, SPT, BL], BF16,
                                             tag="xgblk")
              if nt + 1 < NTT and sp % 4 == 0:
                  emit_p2_group(nxt_blk, nt + 1, sp // 4)
              if nt >= 1 and rep == rec_repeat - 1:
                  # attention for the previous (finished) token tile
                  if sp in (2, 10, 18, 26):
                      if sp == 2:
                          attn_sq = []
                      emit_attn_group(nt - 1, (sp - 2) // 8, attn_sq)
                  elif sp == 34:
                      emit_attn_proj(nt - 1, attn_sq)
              xg_t = cur_blk[:, :, s % SPT, :]
              # 4 gate groups, issued f -> i -> g -> o so each gate's
              # descale+activation starts right after its own 16 MMs; the
              # c-chain overlaps the o MMs and the tail is just
              # sigmoid(o) || tanh(c) -> h-mul.
              ps_f = ps3pool.tile([128, 4, BL], F32, tag="ps_f")
              ps_i = ps3pool.tile([128, 4, BL], F32, tag="ps_i")
              ps_g = ps3pool.tile([128, 4, BL], F32, tag="ps_g")
              ps_o = ps3pool.tile([128, 4, BL], F32, tag="ps_o")
              # xg enters each gate's PSUM bank via ONE identity-stationary
              # MM covering the whole [128, 4, BL] region (a single
              # start=True per bank, so no sibling-region has_written
              # corruption); W.h MMs accumulate on top; activations read
              # PSUM directly with the 0.125 descale in the scale.
              DSC = 0.125 if whh_fp8 else 1.0
              for ps, m0 in ((ps_f, 4), (ps_i, 0), (ps_g, 8), (ps_o, 12)):
                  nc.tensor.matmul(ps[:, :, :], identg,
                                   xg_t[:, m0:m0 + 4, :],
                                   start=True, stop=False)
              for ps, m0 in ((ps_f, 4), (ps_i, 0), (ps_g, 8), (ps_o, 12)):
                  for j in range(4):
                      m = m0 + j
                      for k in range(KC):
                          nc.tensor.matmul(ps[:, j, :],
                                           whhT_sb[:, k, m * 128:(m + 1) * 128],
                                           hzero[:, k, :] if free_run
                                           else h_prev[:, k, :],
                                           start=False, stop=(k == KC - 1))
              gf = gpool3.tile([128, 4, BL], F32, tag="gf")
              nc.scalar.activation(gf, ps_f, SIG, scale=DSC)
              gi = gpool3.tile([128, 4, BL], F32, tag="gi")
              nc.scalar.activation(gi, ps_i, SIG, scale=DSC)
              nc.vector.tensor_mul(cT, gf, cT)        # f*c during g/o MMs
              gg = gpool3.tile([128, 4, BL], F32, tag="gg")
              nc.scalar.activation(gg, ps_g, TANH, scale=DSC)
              go = gpool3.tile([128, 4, BL], F32, tag="go")
              ig = tpool.tile([128, 4, BL], F32, tag="ig")
              nc.vector.tensor_mul(ig, gi, gg)
              nc.vector.tensor_add(cT, cT, ig)
              nc.scalar.activation(go, ps_o, SIG, scale=DSC)  # before tanh_c
              th = tpool.tile([128, 4, BL], F32, tag="th")
              nc.scalar.activation(th, cT, TANH)
              h_t = hpool.tile([128, KC, BL], BF16, tag="h")
              nc.vector.tensor_mul(h_t, go, th)
              nc.vector.tensor_scalar_max(
                  hr_all[:, :, s * BL:(s + 1) * BL], h_t, 0.0)
              h_prev = h_t
          if rec_repeat > 0 and rep == rec_repeat - 1:
              # last token tile's attention (no following block to hide in)
              attn_sq = []
              for mo in range(KC):
                  emit_attn_group(NTT - 1, mo, attn_sq)
              emit_attn_proj(NTT - 1, attn_sq)

    # ================= Phase 4: softmax + ctx + FC ===============
    with nc.named_scope("p4_attn"), \
         tc.tile_pool(name="p4", bufs=4) as p4pool, \
         tc.tile_pool(name="wh", bufs=1) as whpool, \
         tc.tile_pool(name="ps4b", bufs=2, space="PSUM") as ps4bpool:
      for _p4rep in range(p4_repeat):
        # softmax over sequence, per batch element
        nc.sync.dma_start(scr_dram.rearrange("(o t) -> o t", o=1), scores_sb)
        sc_bs = p4pool.tile([BL, S], F32)
        nc.sync.dma_start(sc_bs, scr_dram.rearrange("(s b) -> b s", b=BL))
        mx = p4pool.tile([BL, 1], F32)
        nc.vector.tensor_reduce(mx, sc_bs, axis=AX_X, op=ALU.max)
        nc.vector.tensor_scalar_mul(mx, mx, -1.0)
        at = p4pool.tile([BL, S], F32)
        nc.scalar.activation(at, sc_bs, EXP, bias=mx[:, 0:1], scale=1.0)
        sm = p4pool.tile([BL, 1], F32)
        nc.vector.tensor_reduce(sm, at, axis=AX_X, op=ALU.add)
        nc.vector.reciprocal(sm, sm)
        nc.vector.tensor_scalar_mul(at, at, sm)
        nc.sync.dma_start(attn_dram.rearrange("(s b) -> b s", b=BL), at)
        attn_bc = whpool.tile([128, NTOK], F32, tag="abc")
        nc.sync.dma_start(
            attn_bc,
            bass.AP(tensor=attn_dram.tensor, offset=0, ap=[[0, 128], [1, NTOK]]))

        # ctx = sum_s attn * relu(h)
        for ch in range(KC):
            wh = whpool.tile([128, NTOK], F32, tag="wh")
            nc.vector.tensor_mul(wh, hr_all[:, ch, :], attn_bc)
            nc.vector.tensor_reduce(ctxT_sb[:, ch, :],
                                    wh.rearrange("p (s b) -> p b s", b=BL),
                                    axis=AX_X, op=ALU.add)

        # logits + softmax
        psL = ps4bpool.tile([BL, C], F32)
        for ch in range(KC):
            nc.tensor.matmul(psL, ctxT_sb[:, ch, :], fcwT_sb[:, ch, :],
                             start=(ch == 0), stop=(ch == KC - 1))
        lg = p4pool.tile([BL, C], F32)
        nc.vector.tensor_add(lg, psL, fcb_bc)
        mx2 = p4pool.tile([BL, 1], F32)
        nc.vector.tensor_reduce(mx2, lg, axis=AX_X, op=ALU.max)
        nc.vector.tensor_scalar_mul(mx2, mx2, -1.0)
        pe = p4pool.tile([BL, C], F32)
        nc.scalar.activation(pe, lg, EXP, bias=mx2[:, 0:1], scale=1.0)
        sm2 = p4pool.tile([BL, 1], F32)
        nc.vector.tensor_reduce(sm2, pe, axis=AX_X, op=ALU.add)
        nc.vector.reciprocal(sm2, sm2)
        nc.vector.tensor_scalar_mul(pe, pe, sm2)
        nc.sync.dma_start(out_d, pe)
    _stack.close()


def build_nc(S=S, V=V, **bkw):
    nc = bacc.Bacc("TRN2", target_bir_lowering=False, debug=False,
                   num_devices=NCORES)
    NTOK = S * BL
    whh_dt = mybir.dt.float8e4 if bkw.get("whh_fp8") else BF16
    io = {
        "idx": nc.dram_tensor("idx", [NTOK], I32, kind="ExternalInput").ap(),
        "embed": nc.dram_tensor("embed", [V, D], BF16, kind="ExternalInput").ap(),
        "wihT": nc.dram_tensor("wihT", [D, G4], BF16, kind="ExternalInput").ap(),
        "whhT": nc.dram_tensor("whhT", [Co, G4], whh_dt, kind="ExternalInput").ap(),
        "biasg": nc.dram_tensor("biasg", [G4], F32, kind="ExternalInput").ap(),
        "wword": nc.dram_tensor("wword", [Co, Co], BF16, kind="ExternalInput").ap(),
        "bword": nc.dram_tensor("bword", [Co], F32, kind="ExternalInput").ap(),
        "wproj": nc.dram_tensor("wproj", [Co, 1], BF16, kind="ExternalInput").ap(),
        "fcwT": nc.dram_tensor("fcwT", [Co, C], F32, kind="ExternalInput").ap(),
        "fcb": nc.dram_tensor("fcb", [C], F32, kind="ExternalInput").ap(),
        "probs": nc.dram_tensor("probs", [BL, C], F32, kind="ExternalOutput").ap(),
    }
    with tile.TileContext(nc) as tc:
        build_body(tc, io, S=S, V=V, **bkw)
    nc.compile()
    return nc


def host_prep(inputs, whh_fp8=False):
    """Cast/transpose parameters on host; build per-core in_maps."""
    bf = ml_dtypes.bfloat16
    x = np.asarray(inputs["x"])
    common = {
        "embed": np.ascontiguousarray(np.asarray(inputs["embed"]).astype(bf)),
        "wihT": np.ascontiguousarray(np.asarray(inputs["W_ih"]).T.astype(bf)),
        "whhT": (np.ascontiguousarray((np.asarray(inputs["W_hh"]).T * 8.0).astype(ml_dtypes.float8_e4m3fn))
                  if whh_fp8 else
                  np.ascontiguousarray(np.asarray(inputs["W_hh"]).T.astype(bf))),
        "biasg": np.ascontiguousarray(
            (np.asarray(inputs["b_ih"]) + np.asarray(inputs["b_hh"])).astype(np.float32)),
        "wword": np.ascontiguousarray(np.asarray(inputs["weight_word"]).astype(bf)),
        "bword": np.ascontiguousarray(np.asarray(inputs["bias_word"])[:, 0].astype(np.float32)),
        "wproj": np.ascontiguousarray(np.asarray(inputs["weight_proj_word"]).astype(bf)),
        "fcwT": np.ascontiguousarray(np.asarray(inputs["fc_w"]).T.astype(np.float32)),
        "fcb": np.ascontiguousarray(np.asarray(inputs["fc_b"]).astype(np.float32)),
    }
    in_maps = []
    for c in range(NCORES):
        shard = x[c * BL:(c + 1) * BL, :]          # [BL, S]
        idx = np.ascontiguousarray(shard.T.reshape(-1).astype(np.int32))  # s-major
        in_maps.append({"idx": idx, **common})
    return in_maps


_NC_CACHE = {}
WHH_FP8 = True


def _get_nc():
    if "nc" not in _NC_CACHE:
        _NC_CACHE["nc"] = build_nc(whh_fp8=WHH_FP8)
    return _NC_CACHE["nc"]


def kernel(**inputs):
    nc = _get_nc()
    in_maps = host_prep(inputs, whh_fp8=WHH_FP8)
    res = run_bass_kernel_spmd(nc, in_maps, core_ids=list(range(NCORES)))
    probs = np.concatenate([res.results[c]["probs"] for c in range(NCORES)], axis=0)
    return probs.astype(np.float32)


def run_traced(inputs):
    """Like kernel() but with NTFF tracing; returns (probs, BassKernelResults)."""
    nc = _get_nc()
    in_maps = host_prep(inputs, whh_fp8=WHH_FP8)
    res = run_bass_kernel_spmd(nc, in_maps, core_ids=list(range(NCORES)),
                               trace=True)
    probs = np.concatenate([res.results[c]["probs"] for c in range(NCORES)], axis=0)
    return probs.astype(np.float32), res

